# revision 8
# baseline (speedup 1.0000x reference)
"""Trainium2 Bass kernel v2: 2-layer GCN encoder on 8 NeuronCores.

Optimizations over v1:
- Host precomputes the symmetric norm per edge (no on-device deg pass,
  no dis scalings).
- BN0 folded into W1 on host; the resulting constant-row term handled
  via a rank-1 (s_dst x r) thin matmul in the aggregation PSUM.
- b1/b2 dropped (BatchNorm right after each conv is shift-invariant).
- bf16 everywhere (2x DVE for the one-hot build, 1-pass PE matmuls).
- dma_gather desc-gen parallelized over 4 SWDGE queues with
  prepare_only + trigger_dma (desc-gen was 65% of runtime).
- Variable per-window chunk counts (uniform across cores = per-window
  max) instead of a global CA/CB max.
"""
import sys

if "/opt/trn_rl_repo" not in sys.path:
    sys.path.insert(0, "/opt/trn_rl_repo")

import numpy as np
import ml_dtypes

BF = ml_dtypes.bfloat16

N = 50000
NC = 8
P = 128
NPC = 6250
WINS = 49
SLOTS = WINS * P        # 6272
NTBL = NC * SLOTS       # 50176
EPS = 1e-5
A_HI = 32768
B_LO = NTBL - 32768     # 17408
G = 7                   # windows per gather batch
NB = WINS // G
NQ = 4                  # SWDGE queues


def q_of(i):
    return (i // NPC) * SLOTS + (i % NPC)


def wrap_idx16(idx_flat):
    n = len(idx_flat)
    assert n % 16 == 0
    base = idx_flat.reshape(n // 16, 16).T.astype(np.int16)
    return np.tile(base, (8, 1))


def preprocess(edge_index, edge_weight):
    """Chunk plan (uniform across cores) + per-core tensors."""
    src = np.asarray(edge_index[0], np.int64)
    dst = np.asarray(edge_index[1], np.int64)
    w = np.asarray(edge_weight, np.float32)
    loop = np.arange(N, dtype=np.int64)
    src = np.concatenate([src, loop])
    dst = np.concatenate([dst, loop])
    w = np.concatenate([w, np.ones(N, np.float32)])

    deg = np.bincount(dst, weights=w.astype(np.float64), minlength=N)
    dis = np.where(deg > 0, 1.0 / np.sqrt(np.maximum(deg, 1e-12)), 0.0)
    norm = (dis[src] * w * dis[dst]).astype(np.float32)

    qsrc = q_of(src)
    core = dst // NPC
    wloc = (dst % NPC) // P
    dloc = (dst % NPC) % P

    svec_all = np.zeros((NC, WINS, P), np.float32)
    np.add.at(svec_all, (core, wloc, dloc), norm)

    percore_raw = []
    nA = np.zeros((NC, WINS), np.int64)
    nB = np.zeros((NC, WINS), np.int64)
    for c in range(NC):
        m = core == c
        qs_c, dl_c, nm_c, wl_c = qsrc[m], dloc[m], norm[m], wloc[m]
        isA = (qs_c % 2) == 0          # parity region: even rows
        o = np.lexsort((qs_c, ~isA, wl_c))
        qs_c, dl_c, nm_c, wl_c, isA = (
            qs_c[o], dl_c[o], nm_c[o], wl_c[o], isA[o])
        percore_raw.append((qs_c, dl_c, nm_c, wl_c, isA))
        np.add.at(nA[c], wl_c[isA], 1)
        np.add.at(nB[c], wl_c[~isA], 1)

    ccA = np.maximum(1, -(-nA.max(axis=0) // P)).astype(int)   # [WINS]
    ccB = np.maximum(1, -(-nB.max(axis=0) // P)).astype(int)

    # global chunk list, window-major; key (wi, reg, k) with region-local k
    chunk_list = []
    for wi in range(WINS):
        for k in range(ccA[wi]):
            chunk_list.append((wi, 0, k))
        for k in range(ccB[wi]):
            chunk_list.append((wi, 1, k))
    TC = len(chunk_list)

    # cumulative chunk offsets per window
    win_off = {}
    off = 0
    for wi in range(WINS):
        win_off[wi] = off
        off += ccA[wi] + ccB[wi]
    assert off == TC

    # batches: split each batch's chunks into NQ near-equal contiguous
    # groups; each group -> 1-2 calls (per contiguous region run)
    batches = []
    cmap = {}
    qoff = [[0] * (NB + 1) for _ in range(NQ)]   # chunk offset per queue
    for b in range(NB):
        wlo, whi = b * G, (b + 1) * G
        bchunks = sorted((t for t in chunk_list if wlo <= t[0] < whi),
                         key=lambda t: (t[1], t[0], t[2]))
        echunks = [t for t in bchunks if t[1] == 0]
        ochunks = [t for t in bchunks if t[1] == 1]
        ne, no = len(echunks), len(ochunks)
        groups = [echunks[: (ne + 1) // 2], echunks[(ne + 1) // 2 :],
                  ochunks[: (no + 1) // 2], ochunks[(no + 1) // 2 :]]
        qcalls = []
        for q in range(NQ):
            assert groups[q], f"empty queue group b={b} q={q}"
            qcalls.append([groups[q]])
            col = 0
            for t in groups[q]:
                cmap[t] = (q, b, col)
                col += 1
            qoff[q][b + 1] = qoff[q][b] + col
        batches.append(dict(qcalls=qcalls))

    plan = dict(ccA=ccA, ccB=ccB, TC=TC, batches=batches, cmap=cmap,
                win_off=win_off, qoff=qoff,
                qtot=[qoff[q][NB] for q in range(NQ)])

    percore = []
    for c in range(NC):
        qs_c, dl_c, nm_c, wl_c, isA_c = percore_raw[c]
        idx_by = {}
        for wi in range(WINS):
            mw = wl_c == wi
            for reg in (0, 1):
                mr = mw & (isA_c if reg == 0 else ~isA_c)
                idx_by[(wi, reg)] = (qs_c[mr], dl_c[mr], nm_c[mr])

        # meta: [all dl cols | all w cols], f32 (ACT scale requires FP32)
        meta = np.zeros((P, 2 * TC), np.float32)
        qidx = [[] for _ in range(NQ)]   # (b, col, idx_vec[128])
        for wi in range(WINS):
            cc = ccA[wi] + ccB[wi]
            o = win_off[wi]
            for reg, cck, base_k in ((0, ccA[wi], 0), (1, ccB[wi], ccA[wi])):
                qs, dl, nm = idx_by[(wi, reg)]
                for k in range(cck):
                    lo, hi = k * P, min((k + 1) * P, len(qs))
                    nk = max(0, hi - lo)
                    kk = base_k + k
                    if nk > 0:
                        meta[:nk, o + kk] = dl[lo:hi]
                        meta[:nk, TC + o + kk] = nm[lo:hi]
                    iv = np.zeros(P, np.int64)
                    if nk > 0:
                        iv[:nk] = qs[lo:hi] // 2
                    q, b, col = cmap[(wi, reg, k)]
                    qidx[q].append((b, col, iv))

        qidx16 = []
        for q in range(NQ):
            qidx[q].sort(key=lambda t: (t[0], t[1]))
            assert len(qidx[q]) == plan["qtot"][q]
            flat = np.concatenate([t[2] for t in qidx[q]])
            qidx16.append(wrap_idx16(flat))

        percore.append(dict(meta=meta.astype(BF), qidx16=qidx16,
                            svec=svec_all[c].reshape(1, WINS * P)))
    # batch chunk offsets (global chunk order is window-major => contiguous
    # per batch)
    boff = [0] * (NB + 1)
    t = 0
    for b in range(NB):
        nb_ = sum(ccA[wi] + ccB[wi] for wi in range(b * G, (b + 1) * G))
        boff[b + 1] = boff[b] + nb_
    plan["boff"] = boff
    return plan, percore


def build_in_maps(inputs, plan, percore):
    h = np.asarray(inputs["h"], np.float32)
    m0 = h.mean(axis=0)
    v0 = h.var(axis=0)
    a0 = np.asarray(inputs["g0"], np.float32) / np.sqrt(v0 + EPS)
    c0 = np.asarray(inputs["be0"], np.float32) - m0 * a0
    W1 = np.asarray(inputs["W1"], np.float32)
    W1f = (a0[:, None] * W1).astype(BF)
    rrow = (c0 @ W1).astype(np.float32)

    vecs = np.zeros((1, 8 * P), np.float32)
    vecs[0, 0*P:1*P] = np.asarray(inputs["g1"], np.float32)
    vecs[0, 1*P:2*P] = np.asarray(inputs["be1"], np.float32)
    vecs[0, 2*P:3*P] = np.asarray(inputs["g2"], np.float32)
    vecs[0, 3*P:4*P] = np.asarray(inputs["be2"], np.float32)
    vecs[0, 4*P:5*P] = np.asarray(inputs["bmu"], np.float32)
    vecs[0, 5*P:6*P] = np.asarray(inputs["bls"], np.float32)
    vecs[0, 6*P:7*P] = rrow

    W2 = np.asarray(inputs["W2"], np.float32).astype(BF)
    Wmu = np.asarray(inputs["Wmu"], np.float32).astype(BF)
    Wls = np.asarray(inputs["Wls"], np.float32).astype(BF)

    in_maps = []
    for c in range(NC):
        d = percore[c]
        hown = np.zeros((SLOTS, 5), np.float32)
        hown[:NPC] = h[c * NPC : (c + 1) * NPC]
        in_maps.append({
            "hownT": np.ascontiguousarray(hown.T).astype(BF),
            "meta": d["meta"],
            "idxQ0": d["qidx16"][0],
            "idxQ1": d["qidx16"][1],
            "idxQ2": d["qidx16"][2],
            "idxQ3": d["qidx16"][3],
            "W1f": W1f,
            "W2": W2,
            "Wmu": Wmu,
            "Wls": Wls,
            "vecs": vecs,
            "svecs": d["svec"],
        })
    return in_maps


def build_kernel(plan, debug=False, stage=5):
    import concourse.bacc as bacc
    import concourse.tile as tile
    from concourse import mybir
    from concourse.masks import make_identity

    f32 = mybir.dt.float32
    bf16 = mybir.dt.bfloat16
    i16 = mybir.dt.int16
    AOT = mybir.AluOpType

    ccA, ccB = plan["ccA"], plan["ccB"]
    TC = plan["TC"]
    batches = plan["batches"]
    cmap = plan["cmap"]
    win_off = plan["win_off"]
    qoff = plan["qoff"]
    qtot = plan["qtot"]
    CM = int(max(ccA[wi] + ccB[wi] for wi in range(WINS)))

    nc = bacc.Bacc("TRN2", num_devices=NC, num_swdge_queues=NQ)

    hownT_d = nc.dram_tensor("hownT", [5, SLOTS], bf16, kind="ExternalInput")
    meta_d = nc.dram_tensor("meta", [P, 2 * TC], bf16, kind="ExternalInput")
    idx_d = [nc.dram_tensor(f"idxQ{q}", [P, qtot[q] * 8], i16,
                            kind="ExternalInput") for q in range(NQ)]
    W1f_d = nc.dram_tensor("W1f", [5, P], bf16, kind="ExternalInput")
    W2_d = nc.dram_tensor("W2", [P, P], bf16, kind="ExternalInput")
    Wmu_d = nc.dram_tensor("Wmu", [P, P], bf16, kind="ExternalInput")
    Wls_d = nc.dram_tensor("Wls", [P, P], bf16, kind="ExternalInput")
    vecs_d = nc.dram_tensor("vecs", [1, 8 * P], f32, kind="ExternalInput")
    svecs_d = nc.dram_tensor("svecs", [1, WINS * P], f32, kind="ExternalInput")
    mu_d = nc.dram_tensor("mu_out", [SLOTS, P], f32, kind="ExternalOutput")
    ls_d = nc.dram_tensor("ls_out", [SLOTS, P], f32, kind="ExternalOutput")
    if debug:
        dbg_out1_d = nc.dram_tensor("dbg_out1", [SLOTS, P], f32,
                                    kind="ExternalOutput")

    class StopStage(Exception):
        pass

    with tile.TileContext(nc) as tc:
        with (
            tc.tile_pool(name="const", bufs=1) as cp,
            tc.tile_pool(name="store", bufs=1) as st,
            tc.tile_pool(name="work", bufs=3) as wk,
            tc.tile_pool(name="spool", bufs=3) as sp,
            tc.tile_pool(name="gbuf", bufs=2) as gb,
            tc.tile_pool(name="psum", bufs=2, space="PSUM") as ps,
            tc.tile_pool(name="dram", bufs=1, space="DRAM") as dr,
        ):
          try:
            # ---------- constants ----------
            iota_big = cp.tile([P, CM, P], bf16)
            nc.gpsimd.iota(iota_big[:], pattern=[[0, CM], [1, P]], base=0,
                           channel_multiplier=0,
                           allow_small_or_imprecise_dtypes=True)
            ident = cp.tile([P, P], bf16)
            make_identity(nc, ident[:])
            ones_bf = cp.tile([P, 1], bf16)
            nc.gpsimd.memset(ones_bf[:], 1.0)
            one_row = cp.tile([1, P], f32)
            nc.gpsimd.memset(one_row[:], 1.0)
            one_row_bf = cp.tile([1, P], bf16)
            nc.gpsimd.memset(one_row_bf[:], 1.0)

            meta_t = cp.tile([P, 2 * TC], bf16)
            nc.sync.dma_start(meta_t[:], meta_d[:])
            idx_t = []
            for q in range(NQ):
                t = cp.tile([P, qtot[q] * 8], i16, name=f"idxt{q}")
                nc.sync.dma_start(t[:], idx_d[q][:])
                idx_t.append(t)
            W1f_t = cp.tile([5, P], bf16)
            nc.sync.dma_start(W1f_t[:], W1f_d[:])
            W2_t = cp.tile([P, P], bf16)
            nc.sync.dma_start(W2_t[:], W2_d[:])
            Wmu_t = cp.tile([P, P], bf16)
            nc.sync.dma_start(Wmu_t[:], Wmu_d[:])
            Wls_t = cp.tile([P, P], bf16)
            nc.sync.dma_start(Wls_t[:], Wls_d[:])
            vecs_t = cp.tile([1, 8 * P], f32)
            nc.sync.dma_start(vecs_t[:], vecs_d[:])
            svecs_t = cp.tile([1, WINS * P], f32)
            nc.sync.dma_start(svecs_t[:], svecs_d[:])


            rrow_bf = cp.tile([1, P], bf16)
            nc.vector.tensor_copy(rrow_bf[:], vecs_t[0:1, 6*P:7*P])
            svec_bf = cp.tile([1, WINS * P], bf16)
            nc.vector.tensor_copy(svec_bf[:], svecs_t[:])

            dma_sems = [nc.alloc_semaphore(f"gsem{i}") for i in range(8)]
            prep_ctr = [0]
            sem_counts = [0] * 8

            # ---------- helpers ----------
            def transform(src_bf, rhs_list):
                kdim = src_bf.shape[-1]
                tps = ps.tile([P, P], bf16, space="PSUM", tag="tpsT")
                nc.tensor.transpose(tps[:kdim, :], src_bf, ident[:])
                tsb = wk.tile([P, P], bf16, tag="tsb")
                nc.vector.tensor_copy(tsb[:kdim, :], tps[:kdim, :])
                outs = []
                for rhs in rhs_list:
                    mps = ps.tile([P, P], f32, space="PSUM", tag="tps")
                    nc.tensor.matmul(mps[:], lhsT=tsb[:kdim, :], rhs=rhs,
                                     start=True, stop=True)
                    outs.append(mps)
                return outs

            pending_bufs = {}

            def emit_preps(tblv, key, b):
                bat = batches[b]
                bufq = []
                for q in range(NQ):
                    ncols = qoff[q][b + 1] - qoff[q][b]
                    buf = gb.tile([P, max(ncols, 1), P], bf16, tag=f"buf{q}")
                    cstart = qoff[q][b]
                    (call,) = bat["qcalls"][q]
                    ncall = len(call)
                    reg = call[0][1]
                    src = tblv[:, reg * P : (reg + 1) * P]
                    nc.gpsimd.dma_gather(
                        buf[:, 0:ncall, :], src,
                        idx_t[q][:, cstart * 8 : (cstart + ncall) * 8],
                        ncall * P, ncall * P, P,
                        elem_step=2 * P,
                        single_packet=False, prepare_only=True,
                        sem=dma_sems[prep_ctr[0] % 8], queue_num=q)
                    sem_counts[prep_ctr[0] % 8] += 1
                    prep_ctr[0] += 1
                    bufq.append(buf)
                pending_bufs[(key, b)] = (bufq, [s for s in sem_counts])

            # ---------- z1 table ----------
            ag_in1 = dr.tile([SLOTS, P], bf16)
            tbl1 = dr.tile([NTBL, P], bf16, addr_space="Shared")
            h_all = sp.tile([5, SLOTS], bf16, tag="hall", bufs=1)
            nc.sync.dma_start(h_all[:], hownT_d[:])
            for b in range(NB):
                zchunk = wk.tile([P, G, P], bf16, tag="zchunk", bufs=2)
                for j in range(G):
                    wi = b * G + j
                    zps = ps.tile([P, P], f32, space="PSUM", tag="tps")
                    nc.tensor.matmul(zps[:], lhsT=h_all[:, wi*P:(wi+1)*P],
                                     rhs=W1f_t[:], start=True, stop=True)
                    nc.vector.tensor_copy(zchunk[:, j, :], zps[:])
                nc.sync.dma_start(
                    ag_in1[b * G * P : (b + 1) * G * P, :].rearrange(
                        "(j p) d -> p j d", p=P),
                    zchunk[:])
            nc.gpsimd.collective_compute(
                "AllGather", AOT.bypass, replica_groups=[list(range(NC))],
                ins=[ag_in1[:]], outs=[tbl1[:]])

            if stage < 2:
                raise StopStage

            # ---------- aggregation ----------
            out_store = st.tile([P, WINS, P], bf16)

            ACTF = mybir.ActivationFunctionType

            def s_build(wi):
                """Batched one-hot on DVE, per-chunk weight scale on ACT."""
                cc = int(ccA[wi] + ccB[wi])
                o = win_off[wi]
                dlb = meta_t[:, o : o + cc].rearrange(
                    "p (k x) -> p k x", x=1).broadcast_to([P, cc, P])
                tmp = sp.tile([P, CM, P], bf16, tag="stmp")
                nc.vector.tensor_tensor(tmp[:, :cc, :], iota_big[:, :cc, :],
                                        dlb, op=AOT.is_equal)
                wb = meta_t[:, TC + o : TC + o + cc].rearrange(
                    "p (k x) -> p k x", x=1).broadcast_to([P, cc, P])
                s_t = sp.tile([P, CM, P], bf16, tag="s")
                nc.vector.tensor_tensor(s_t[:, :cc, :], tmp[:, :cc, :], wb,
                                        op=AOT.mult)
                return s_t

            def agg_pass(tbl, layer, key):
                wait_marks = [-1] * 8
                sum_acc = wk.tile([1, P], f32, tag="sacc")
                sq_acc = wk.tile([1, P], f32, tag="qacc")
                nc.gpsimd.memset(sum_acc[:], 0.0)
                nc.gpsimd.memset(sq_acc[:], 0.0)
                tblv = tbl[:, :].rearrange("(a b) d -> a (b d)", b=2)
                for b, bat in enumerate(batches):
                    if (key, b) not in pending_bufs:
                        emit_preps(tblv, key, b)
                    bufq, marks = pending_bufs.pop((key, b))
                    for q in range(NQ):
                        nc.gpsimd.trigger_dma(count=1, queue_num=q)
                    # explicit completion waits on our own gather sems (the
                    # Tile DMASW bridge releases consumers too early on HW)
                    for s in range(8):
                        if marks[s] > wait_marks[s]:
                            nc.tensor.wait_ge(dma_sems[s], 16 * marks[s])
                            wait_marks[s] = marks[s]
                    for wi in range(b * G, (b + 1) * G):
                        cc = int(ccA[wi] + ccB[wi])
                        agg = ps.tile([P, P], f32, space="PSUM", tag="agg",
                                      bufs=3)
                        s_t = s_build(wi)
                        nci = 0
                        ntot = cc + (1 if layer == 1 else 0)
                        for reg, cck, base_k in (
                            (0, int(ccA[wi]), 0),
                            (1, int(ccB[wi]), int(ccA[wi])),
                        ):
                            for k in range(cck):
                                kk = base_k + k
                                q, bb, col = cmap[(wi, reg, k)]
                                assert bb == b
                                nc.tensor.matmul(
                                    agg[:], lhsT=s_t[:, kk, :],
                                    rhs=bufq[q][:, col, :],
                                    start=(nci == 0), stop=(nci == ntot - 1))
                                nci += 1
                        if layer == 1:
                            # correction LAST so no agg-group matmul precedes
                            # the gather deps on the in-order PE queue (the
                            # scheduler would otherwise hoist it before the
                            # z-table transforms the AllGather needs).
                            nc.tensor.matmul(
                                agg[:], lhsT=svec_bf[0:1, wi*P:(wi+1)*P],
                                rhs=rrow_bf[:], start=False, stop=True)
                            nci += 1
                        outw = out_store[:, wi, :]
                        nc.vector.tensor_copy(outw, agg[:])
                        sq = wk.tile([P, P], bf16, tag="sq")
                        nc.scalar.square(sq[:], outw)
                        sps = ps.tile([1, P], f32, space="PSUM", tag="sps",
                                      bufs=1)
                        nc.tensor.matmul(sps[:], lhsT=ones_bf[:], rhs=outw,
                                         start=True, stop=True)
                        nc.vector.tensor_tensor(sum_acc[:], sum_acc[:],
                                                sps[:], op=AOT.add)
                        qps = ps.tile([1, P], f32, space="PSUM", tag="sps",
                                      bufs=1)
                        nc.tensor.matmul(qps[:], lhsT=ones_bf[:], rhs=sq[:],
                                         start=True, stop=True)
                        nc.vector.tensor_tensor(sq_acc[:], sq_acc[:],
                                                qps[:], op=AOT.add)
                return sum_acc, sq_acc

            def bn_reduce(sum_acc, sq_acc, g_row, be_row, name):
                bn_in = dr.tile([1, 2 * P], f32, name=f"bnin_{name}")
                bn_out = dr.tile([1, 2 * P], f32, addr_space="Shared",
                                 name=f"bnout_{name}")
                pack = wk.tile([1, 2 * P], f32, tag="bnpack")
                nc.vector.tensor_copy(pack[0:1, 0:P], sum_acc[:])
                nc.vector.tensor_copy(pack[0:1, P : 2 * P], sq_acc[:])
                nc.sync.dma_start(bn_in[:], pack[:])
                nc.gpsimd.collective_compute(
                    "AllReduce", AOT.add, replica_groups=[list(range(NC))],
                    ins=[bn_in[:]], outs=[bn_out[:]])
                bn_t = wk.tile([1, 2 * P], f32, tag="bnt")
                nc.sync.dma_start(bn_t[:], bn_out[:])
                mean = wk.tile([1, P], f32, tag="bn1")
                nc.vector.tensor_scalar(mean[:], bn_t[0:1, 0:P], 1.0 / N,
                                        None, op0=AOT.mult)
                var = wk.tile([1, P], f32, tag="bn2")
                nc.vector.tensor_scalar(var[:], bn_t[0:1, P : 2 * P], 1.0 / N,
                                        None, op0=AOT.mult)
                msq = wk.tile([1, P], f32, tag="bn3")
                nc.vector.tensor_tensor(msq[:], mean[:], mean[:], op=AOT.mult)
                nc.vector.tensor_tensor(var[:], var[:], msq[:],
                                        op=AOT.subtract)
                nc.vector.tensor_scalar(var[:], var[:], EPS, None, op0=AOT.add)
                rc = wk.tile([1, P], f32, tag="bn3")
                nc.vector.reciprocal(rc[:], var[:])
                rs = wk.tile([1, P], f32, tag="bn3")
                nc.scalar.sqrt(rs[:], rc[:])
                a_row = wk.tile([1, P], f32, tag="bn4")
                nc.vector.tensor_tensor(a_row[:], rs[:], g_row, op=AOT.mult)
                c_row = wk.tile([1, P], f32, tag="bn5")
                nc.vector.tensor_tensor(c_row[:], mean[:], a_row[:],
                                        op=AOT.mult)
                nc.vector.tensor_tensor(c_row[:], be_row, c_row[:],
                                        op=AOT.subtract)
                a_bf = wk.tile([1, P], bf16, tag="bn6")
                nc.vector.tensor_copy(a_bf[:], a_row[:])
                c_bf = wk.tile([1, P], bf16, tag="bn7")
                nc.vector.tensor_copy(c_bf[:], c_row[:])
                af_ps = ps.tile([P, P], f32, space="PSUM", tag="tps")
                nc.tensor.matmul(af_ps[:], lhsT=one_row_bf[:], rhs=a_bf[:],
                                 start=True, stop=True)
                a_full = st.tile([P, P], bf16, name=f"afull_{name}")
                nc.vector.tensor_copy(a_full[:], af_ps[:])
                cf_ps = ps.tile([P, P], f32, space="PSUM", tag="tps")
                nc.tensor.matmul(cf_ps[:], lhsT=one_row_bf[:], rhs=c_bf[:],
                                 start=True, stop=True)
                c_full = st.tile([P, P], bf16, name=f"cfull_{name}")
                nc.vector.tensor_copy(c_full[:], cf_ps[:])
                return a_full, c_full

            _wait_base = [0] * 8

            # ----- layer 1 -----
            sum1, sq1 = agg_pass(tbl1, 1, "L1")
            ag_in2 = dr.tile([SLOTS, P], bf16)
            tbl2 = dr.tile([NTBL, P], bf16, addr_space="Shared")

            a1f, c1f = bn_reduce(sum1, sq1, vecs_t[0:1, 0:P],
                                 vecs_t[0:1, P:2*P], "bn1")
            if debug:
                for wi in range(WINS):
                    o32 = wk.tile([P, P], f32, tag="o32")
                    nc.vector.tensor_copy(o32[:], out_store[:, wi, :])
                    nc.sync.dma_start(dbg_out1_d[wi*P:(wi+1)*P, :], o32[:])
            if stage < 3:
                raise StopStage

            # ----- z2 table -----
            for b in range(NB):
                zchunk = wk.tile([P, G, P], bf16, tag="zchunk", bufs=2)
                for j in range(G):
                    wi = b * G + j
                    x1w = wk.tile([P, P], bf16, tag="x1w")
                    nc.vector.tensor_tensor(x1w[:], out_store[:, wi, :],
                                            a1f[:], op=AOT.mult)
                    nc.vector.tensor_tensor(x1w[:], x1w[:], c1f[:],
                                            op=AOT.add)
                    nc.vector.tensor_scalar(x1w[:], x1w[:], 0.0, None,
                                            op0=AOT.max)
                    (w2ps,) = transform(x1w[:], [W2_t[:]])
                    nc.vector.tensor_copy(zchunk[:, j, :], w2ps[:])
                nc.sync.dma_start(
                    ag_in2[b * G * P : (b + 1) * G * P, :].rearrange(
                        "(j p) d -> p j d", p=P),
                    zchunk[:])
            nc.gpsimd.collective_compute(
                "AllGather", AOT.bypass, replica_groups=[list(range(NC))],
                ins=[ag_in2[:]], outs=[tbl2[:]])

            _wait_base = [sem_counts[s] for s in range(8)]

            # ----- layer 2 -----
            sum2, sq2 = agg_pass(tbl2, 2, "L2")
            a2f, c2f = bn_reduce(sum2, sq2, vecs_t[0:1, 2*P:3*P],
                                 vecs_t[0:1, 3*P:4*P], "bn2")
            if stage < 4:
                raise StopStage

            # ----- heads -----
            bmu_bf = cp.tile([1, P], bf16)
            nc.vector.tensor_copy(bmu_bf[:], vecs_t[0:1, 4*P:5*P])
            bls_bf = cp.tile([1, P], bf16)
            nc.vector.tensor_copy(bls_bf[:], vecs_t[0:1, 5*P:6*P])

            for b in range(NB):
                muc = wk.tile([P, G, P], f32, tag="muc", bufs=2)
                lsc = wk.tile([P, G, P], f32, tag="lsc", bufs=2)
                for j in range(G):
                    wi = b * G + j
                    x2w = wk.tile([P, P], bf16, tag="x1w")
                    nc.vector.tensor_tensor(x2w[:], out_store[:, wi, :],
                                            a2f[:], op=AOT.mult)
                    nc.vector.tensor_tensor(x2w[:], x2w[:], c2f[:],
                                            op=AOT.add)
                    nc.vector.tensor_scalar(x2w[:], x2w[:], 0.0, None,
                                            op0=AOT.max)
                    kdim = P
                    tps = ps.tile([P, P], bf16, space="PSUM", tag="tpsT")
                    nc.tensor.transpose(tps[:], x2w[:], ident[:])
                    tsb = wk.tile([P, P], bf16, tag="tsb")
                    nc.vector.tensor_copy(tsb[:], tps[:])
                    mups = ps.tile([P, P], f32, space="PSUM", tag="tps")
                    nc.tensor.matmul(mups[:], lhsT=tsb[:], rhs=Wmu_t[:],
                                     start=True, stop=False)
                    nc.tensor.matmul(mups[:], lhsT=one_row_bf[:],
                                     rhs=bmu_bf[:], start=False, stop=True)
                    lsps = ps.tile([P, P], f32, space="PSUM", tag="tps")
                    nc.tensor.matmul(lsps[:], lhsT=tsb[:], rhs=Wls_t[:],
                                     start=True, stop=False)
                    nc.tensor.matmul(lsps[:], lhsT=one_row_bf[:],
                                     rhs=bls_bf[:], start=False, stop=True)
                    nc.vector.tensor_copy(muc[:, j, :], mups[:])
                    nc.vector.tensor_copy(lsc[:, j, :], lsps[:])
                nc.sync.dma_start(
                    mu_d[b * G * P : (b + 1) * G * P, :].rearrange(
                        "(j p) d -> p j d", p=P),
                    muc[:])
                nc.sync.dma_start(
                    ls_d[b * G * P : (b + 1) * G * P, :].rearrange(
                        "(j p) d -> p j d", p=P),
                    lsc[:])
          except StopStage:
            pass

    nc.compile()
    return nc


def _plan_sig(plan):
    return (tuple(plan["ccA"]), tuple(plan["ccB"]),
            tuple(plan["qtot"]))


_CACHE = {}


def run(inputs, debug=False, trace=False, stage=5):
    import time
    from concourse.bass_utils import run_bass_kernel_spmd

    t0 = time.time()
    plan, percore = preprocess(inputs["edge_index"], inputs["edge_weight"])
    in_maps = build_in_maps(inputs, plan, percore)
    prep_s = time.time() - t0

    t0 = time.time()
    nc = build_kernel(plan, debug=debug, stage=stage)
    build_s = time.time() - t0

    t0 = time.time()
    res = run_bass_kernel_spmd(nc, in_maps, core_ids=list(range(NC)),
                               trace=trace)
    run_s = time.time() - t0
    print(f"[gcn2] prep {prep_s:.1f}s build {build_s:.1f}s run {run_s:.1f}s",
          flush=True)

    mu = np.zeros((N, P), np.float32)
    ls = np.zeros((N, P), np.float32)
    for c in range(NC):
        if "mu_out" in res.results[c]:
            mu[c * NPC : (c + 1) * NPC] = res.results[c]["mu_out"][:NPC]
            ls[c * NPC : (c + 1) * NPC] = res.results[c]["ls_out"][:NPC]
    return (mu, ls), res


def make_pjrt_runner(nc, in_maps):
    import jax
    from jax.sharding import Mesh, PartitionSpec, NamedSharding
    from jax.experimental.shard_map import shard_map
    from concourse import bass2jax, mybir
    from concourse.bass2jax import _bass_exec_p, install_neuronx_cc_hook

    install_neuronx_cc_hook()
    n_cores = len(in_maps)
    partition_name = nc.partition_id_tensor.name if nc.partition_id_tensor else None
    in_names, out_names, out_avals, zero_outs = [], [], [], []
    for alloc in nc.m.functions[0].allocations:
        if not isinstance(alloc, mybir.MemoryLocationSet):
            continue
        name = alloc.memorylocations[0].name
        if alloc.kind == "ExternalInput":
            if name != partition_name:
                in_names.append(name)
        elif alloc.kind == "ExternalOutput":
            shape = tuple(alloc.tensor_shape)
            dt = mybir.dt.np(alloc.dtype)
            out_avals.append(jax.core.ShapedArray(shape, dt))
            out_names.append(name)
            zero_outs.append(np.zeros(shape, dt))
    n_params = len(in_names)
    n_outs = len(out_avals)
    in_names.extend(out_names)
    if partition_name is not None:
        in_names.append(partition_name)

    def _body(*args):
        operands = list(args)
        if partition_name is not None:
            operands.append(bass2jax.partition_id_tensor())
        outs = _bass_exec_p.bind(
            *operands,
            out_avals=tuple(out_avals), in_names=tuple(in_names),
            out_names=tuple(out_names), lowering_input_output_aliases=(),
            sim_require_finite=True, sim_require_nnan=True, nc=nc)
        return tuple(outs)

    devices = jax.devices()[:n_cores]
    mesh = Mesh(np.asarray(devices), ("core",))
    in_specs = (PartitionSpec("core"),) * (n_params + n_outs)
    out_specs = (PartitionSpec("core"),) * len(out_names)
    sharded = jax.jit(
        shard_map(_body, mesh=mesh, in_specs=in_specs, out_specs=out_specs,
                  check_rep=False),
        keep_unused=True)
    sh = NamedSharding(mesh, PartitionSpec("core"))
    per_core = [[np.asarray(m[name]) for name in in_names[:n_params]]
                for m in in_maps]
    concat_in = [
        jax.device_put(
            np.concatenate([per_core[c][i] for c in range(n_cores)], axis=0),
            sh)
        for i in range(n_params)
    ]
    zeros_dev = [jax.device_put(
                     np.zeros((n_cores * z.shape[0], *z.shape[1:]), z.dtype),
                     sh)
                 for z in zero_outs]

    def execute():
        return sharded(*concat_in, *zeros_dev)

    def unpack(out_arrs):
        return [
            {name: np.asarray(out_arrs[i]).reshape(
                n_cores, *out_avals[i].shape)[c]
             for i, name in enumerate(out_names)}
            for c in range(n_cores)
        ]
    return execute, unpack


def run_timed(inputs, iters=8, stage=5):
    import time, jax
    plan, percore = preprocess(inputs["edge_index"], inputs["edge_weight"])
    in_maps = build_in_maps(inputs, plan, percore)
    key = (_plan_sig(plan), stage)
    if key not in _CACHE:
        _CACHE[key] = build_kernel(plan, stage=stage)
    nc = _CACHE[key]
    execute, unpack = make_pjrt_runner(nc, in_maps)
    t0 = time.time()
    out = execute()
    jax.block_until_ready(out)
    t_first = time.time() - t0
    t0 = time.time()
    last = None
    for _ in range(iters):
        last = execute()
    jax.block_until_ready(last)
    t_total = time.time() - t0
    per_iter_ns = t_total / iters * 1e9
    results = unpack(last)
    mu = np.zeros((N, P), np.float32)
    ls = np.zeros((N, P), np.float32)
    for c in range(NC):
        if "mu_out" in results[c]:
            mu[c * NPC : (c + 1) * NPC] = results[c]["mu_out"][:NPC]
            ls[c * NPC : (c + 1) * NPC] = results[c]["ls_out"][:NPC]
    return (mu, ls), per_iter_ns, t_first


def kernel(**inputs):
    from concourse.bass_utils import run_bass_kernel_spmd

    plan, percore = preprocess(inputs["edge_index"], inputs["edge_weight"])
    in_maps = build_in_maps(inputs, plan, percore)
    key = (_plan_sig(plan), 5)
    if key not in _CACHE:
        _CACHE[key] = build_kernel(plan)
    nc = _CACHE[key]
    res = run_bass_kernel_spmd(nc, in_maps, core_ids=list(range(NC)))

    mu = np.zeros((N, P), np.float32)
    ls = np.zeros((N, P), np.float32)
    for c in range(NC):
        mu[c * NPC : (c + 1) * NPC] = res.results[c]["mu_out"][:NPC]
        ls[c * NPC : (c + 1) * NPC] = res.results[c]["ls_out"][:NPC]
    return (mu, ls)


# revision 9
# speedup vs baseline: 1.8506x; 1.8506x over previous
"""Trainium2 Bass kernel v2: 2-layer GCN encoder on 8 NeuronCores.

Optimizations over v1:
- Host precomputes the symmetric norm per edge (no on-device deg pass,
  no dis scalings).
- BN0 folded into W1 on host; the resulting constant-row term handled
  via a rank-1 (s_dst x r) thin matmul in the aggregation PSUM.
- b1/b2 dropped (BatchNorm right after each conv is shift-invariant).
- bf16 everywhere (2x DVE for the one-hot build, 1-pass PE matmuls).
- dma_gather desc-gen parallelized over 4 SWDGE queues with
  prepare_only + trigger_dma (desc-gen was 65% of runtime).
- Variable per-window chunk counts (uniform across cores = per-window
  max) instead of a global CA/CB max.
"""
import sys

if "/opt/trn_rl_repo" not in sys.path:
    sys.path.insert(0, "/opt/trn_rl_repo")

import numpy as np
import ml_dtypes

BF = ml_dtypes.bfloat16

N = 50000
NC = 8
P = 128
NPC = 6250
WINS = 49
SLOTS = WINS * P        # 6272
NTBL = NC * SLOTS       # 50176
EPS = 1e-5
A_HI = 32768
B_LO = NTBL - 32768     # 17408
G = 7                   # windows per gather batch
NB = WINS // G
NQ = 4                  # SWDGE queues


def q_of(i):
    return (i // NPC) * SLOTS + (i % NPC)


def wrap_idx16(idx_flat):
    n = len(idx_flat)
    assert n % 16 == 0
    base = idx_flat.reshape(n // 16, 16).T.astype(np.int16)
    return np.tile(base, (8, 1))


def preprocess(edge_index, edge_weight):
    """Chunk plan (uniform across cores) + per-core tensors."""
    src = np.asarray(edge_index[0], np.int64)
    dst = np.asarray(edge_index[1], np.int64)
    w = np.asarray(edge_weight, np.float32)
    loop = np.arange(N, dtype=np.int64)
    src = np.concatenate([src, loop])
    dst = np.concatenate([dst, loop])
    w = np.concatenate([w, np.ones(N, np.float32)])

    deg = np.bincount(dst, weights=w.astype(np.float64), minlength=N)
    dis = np.where(deg > 0, 1.0 / np.sqrt(np.maximum(deg, 1e-12)), 0.0)
    norm = (dis[src] * w * dis[dst]).astype(np.float32)

    qsrc = q_of(src)
    core = dst // NPC
    wloc = (dst % NPC) // P
    dloc = (dst % NPC) % P

    svec_all = np.zeros((NC, WINS, P), np.float32)
    np.add.at(svec_all, (core, wloc, dloc), norm)

    percore_raw = []
    nA = np.zeros((NC, WINS), np.int64)
    nB = np.zeros((NC, WINS), np.int64)
    for c in range(NC):
        m = core == c
        qs_c, dl_c, nm_c, wl_c = qsrc[m], dloc[m], norm[m], wloc[m]
        isA = (qs_c % 2) == 0          # parity region: even rows
        o = np.lexsort((qs_c, ~isA, wl_c))
        qs_c, dl_c, nm_c, wl_c, isA = (
            qs_c[o], dl_c[o], nm_c[o], wl_c[o], isA[o])
        percore_raw.append((qs_c, dl_c, nm_c, wl_c, isA))
        np.add.at(nA[c], wl_c[isA], 1)
        np.add.at(nB[c], wl_c[~isA], 1)

    ccA = np.maximum(1, -(-nA.max(axis=0) // P)).astype(int)   # [WINS]
    ccB = np.maximum(1, -(-nB.max(axis=0) // P)).astype(int)

    # global chunk list, window-major; key (wi, reg, k) with region-local k
    chunk_list = []
    for wi in range(WINS):
        for k in range(ccA[wi]):
            chunk_list.append((wi, 0, k))
        for k in range(ccB[wi]):
            chunk_list.append((wi, 1, k))
    TC = len(chunk_list)

    # cumulative chunk offsets per window
    win_off = {}
    off = 0
    for wi in range(WINS):
        win_off[wi] = off
        off += ccA[wi] + ccB[wi]
    assert off == TC

    # batches: split each batch's chunks into NQ near-equal contiguous
    # groups; each group -> 1-2 calls (per contiguous region run)
    batches = []
    cmap = {}
    qoff = [[0] * (NB + 1) for _ in range(NQ)]   # chunk offset per queue
    for b in range(NB):
        wlo, whi = b * G, (b + 1) * G
        bchunks = sorted((t for t in chunk_list if wlo <= t[0] < whi),
                         key=lambda t: (t[1], t[0], t[2]))
        echunks = [t for t in bchunks if t[1] == 0]
        ochunks = [t for t in bchunks if t[1] == 1]
        ne, no = len(echunks), len(ochunks)
        groups = [echunks[: (ne + 1) // 2], echunks[(ne + 1) // 2 :],
                  ochunks[: (no + 1) // 2], ochunks[(no + 1) // 2 :]]
        qcalls = []
        for q in range(NQ):
            assert groups[q], f"empty queue group b={b} q={q}"
            qcalls.append([groups[q]])
            col = 0
            for t in groups[q]:
                cmap[t] = (q, b, col)
                col += 1
            qoff[q][b + 1] = qoff[q][b] + col
        batches.append(dict(qcalls=qcalls))

    plan = dict(ccA=ccA, ccB=ccB, TC=TC, batches=batches, cmap=cmap,
                win_off=win_off, qoff=qoff,
                qtot=[qoff[q][NB] for q in range(NQ)])

    percore = []
    for c in range(NC):
        qs_c, dl_c, nm_c, wl_c, isA_c = percore_raw[c]
        idx_by = {}
        for wi in range(WINS):
            mw = wl_c == wi
            for reg in (0, 1):
                mr = mw & (isA_c if reg == 0 else ~isA_c)
                idx_by[(wi, reg)] = (qs_c[mr], dl_c[mr], nm_c[mr])

        # meta: [all dl cols | all w cols], f32 (ACT scale requires FP32)
        meta = np.zeros((P, 2 * TC), np.float32)
        qidx = [[] for _ in range(NQ)]   # (b, col, idx_vec[128])
        for wi in range(WINS):
            cc = ccA[wi] + ccB[wi]
            o = win_off[wi]
            for reg, cck, base_k in ((0, ccA[wi], 0), (1, ccB[wi], ccA[wi])):
                qs, dl, nm = idx_by[(wi, reg)]
                for k in range(cck):
                    lo, hi = k * P, min((k + 1) * P, len(qs))
                    nk = max(0, hi - lo)
                    kk = base_k + k
                    if nk > 0:
                        meta[:nk, o + kk] = dl[lo:hi]
                        meta[:nk, TC + o + kk] = nm[lo:hi]
                    iv = np.zeros(P, np.int64)
                    if nk > 0:
                        iv[:nk] = qs[lo:hi] // 2
                    q, b, col = cmap[(wi, reg, k)]
                    qidx[q].append((b, col, iv))

        qidx16 = []
        for q in range(NQ):
            qidx[q].sort(key=lambda t: (t[0], t[1]))
            assert len(qidx[q]) == plan["qtot"][q]
            flat = np.concatenate([t[2] for t in qidx[q]])
            qidx16.append(wrap_idx16(flat))

        percore.append(dict(meta=meta, qidx16=qidx16,
                            svec=svec_all[c].reshape(1, WINS * P)))
    # batch chunk offsets (global chunk order is window-major => contiguous
    # per batch)
    boff = [0] * (NB + 1)
    t = 0
    for b in range(NB):
        nb_ = sum(ccA[wi] + ccB[wi] for wi in range(b * G, (b + 1) * G))
        boff[b + 1] = boff[b] + nb_
    plan["boff"] = boff
    return plan, percore


def build_in_maps(inputs, plan, percore):
    h = np.asarray(inputs["h"], np.float32)
    m0 = h.mean(axis=0)
    v0 = h.var(axis=0)
    a0 = np.asarray(inputs["g0"], np.float32) / np.sqrt(v0 + EPS)
    c0 = np.asarray(inputs["be0"], np.float32) - m0 * a0
    W1 = np.asarray(inputs["W1"], np.float32)
    W1f = (a0[:, None] * W1).astype(BF)
    rrow = (c0 @ W1).astype(np.float32)

    vecs = np.zeros((1, 8 * P), np.float32)
    vecs[0, 0*P:1*P] = np.asarray(inputs["g1"], np.float32)
    vecs[0, 1*P:2*P] = np.asarray(inputs["be1"], np.float32)
    vecs[0, 2*P:3*P] = np.asarray(inputs["g2"], np.float32)
    vecs[0, 3*P:4*P] = np.asarray(inputs["be2"], np.float32)
    vecs[0, 4*P:5*P] = np.asarray(inputs["bmu"], np.float32)
    vecs[0, 5*P:6*P] = np.asarray(inputs["bls"], np.float32)
    vecs[0, 6*P:7*P] = rrow

    W2 = np.asarray(inputs["W2"], np.float32).astype(BF)
    Wmu = np.asarray(inputs["Wmu"], np.float32).astype(BF)
    Wls = np.asarray(inputs["Wls"], np.float32).astype(BF)

    in_maps = []
    for c in range(NC):
        d = percore[c]
        hown = np.zeros((SLOTS, 5), np.float32)
        hown[:NPC] = h[c * NPC : (c + 1) * NPC]
        in_maps.append({
            "hownT": np.ascontiguousarray(hown.T).astype(BF),
            "meta": d["meta"],
            "idxQ0": d["qidx16"][0],
            "idxQ1": d["qidx16"][1],
            "idxQ2": d["qidx16"][2],
            "idxQ3": d["qidx16"][3],
            "W1f": W1f,
            "W2": W2,
            "Wmu": Wmu,
            "Wls": Wls,
            "vecs": vecs,
            "svecs": d["svec"],
        })
    return in_maps


def build_kernel(plan, debug=False, stage=5):
    import concourse.bacc as bacc
    import concourse.tile as tile
    from concourse import mybir
    from concourse.masks import make_identity

    f32 = mybir.dt.float32
    bf16 = mybir.dt.bfloat16
    i16 = mybir.dt.int16
    AOT = mybir.AluOpType

    ccA, ccB = plan["ccA"], plan["ccB"]
    TC = plan["TC"]
    batches = plan["batches"]
    cmap = plan["cmap"]
    win_off = plan["win_off"]
    qoff = plan["qoff"]
    qtot = plan["qtot"]
    CM = int(max(ccA[wi] + ccB[wi] for wi in range(WINS)))

    nc = bacc.Bacc("TRN2", num_devices=NC, num_swdge_queues=NQ)

    hownT_d = nc.dram_tensor("hownT", [5, SLOTS], bf16, kind="ExternalInput")
    meta_d = nc.dram_tensor("meta", [P, 2 * TC], f32, kind="ExternalInput")
    idx_d = [nc.dram_tensor(f"idxQ{q}", [P, qtot[q] * 8], i16,
                            kind="ExternalInput") for q in range(NQ)]
    W1f_d = nc.dram_tensor("W1f", [5, P], bf16, kind="ExternalInput")
    W2_d = nc.dram_tensor("W2", [P, P], bf16, kind="ExternalInput")
    Wmu_d = nc.dram_tensor("Wmu", [P, P], bf16, kind="ExternalInput")
    Wls_d = nc.dram_tensor("Wls", [P, P], bf16, kind="ExternalInput")
    vecs_d = nc.dram_tensor("vecs", [1, 8 * P], f32, kind="ExternalInput")
    svecs_d = nc.dram_tensor("svecs", [1, WINS * P], f32, kind="ExternalInput")
    mu_d = nc.dram_tensor("mu_out", [SLOTS, P], f32, kind="ExternalOutput")
    ls_d = nc.dram_tensor("ls_out", [SLOTS, P], f32, kind="ExternalOutput")
    if debug:
        dbg_out1_d = nc.dram_tensor("dbg_out1", [SLOTS, P], f32,
                                    kind="ExternalOutput")

    class StopStage(Exception):
        pass

    with tile.TileContext(nc) as tc:
        with (
            tc.tile_pool(name="const", bufs=1) as cp,
            tc.tile_pool(name="store", bufs=1) as st,
            tc.tile_pool(name="work", bufs=3) as wk,
            tc.tile_pool(name="spool", bufs=3) as sp,
            tc.tile_pool(name="gbuf", bufs=2) as gb,
            tc.tile_pool(name="psum", bufs=2, space="PSUM") as ps,
            tc.tile_pool(name="dram", bufs=1, space="DRAM") as dr,
        ):
          try:
            # ---------- constants ----------
            iota_t = cp.tile([P, P], f32)
            nc.gpsimd.iota(iota_t[:], pattern=[[1, P]], base=0,
                           channel_multiplier=0,
                           allow_small_or_imprecise_dtypes=True)
            ident = cp.tile([P, P], bf16)
            make_identity(nc, ident[:])
            ones_bf = cp.tile([P, 1], bf16)
            nc.gpsimd.memset(ones_bf[:], 1.0)
            one_row = cp.tile([1, P], f32)
            nc.gpsimd.memset(one_row[:], 1.0)
            one_row_bf = cp.tile([1, P], bf16)
            nc.gpsimd.memset(one_row_bf[:], 1.0)

            meta_t = cp.tile([P, 2 * TC], f32)
            nc.sync.dma_start(meta_t[:], meta_d[:])
            idx_t = []
            for q in range(NQ):
                t = cp.tile([P, qtot[q] * 8], i16, name=f"idxt{q}")
                nc.sync.dma_start(t[:], idx_d[q][:])
                idx_t.append(t)
            W1f_t = cp.tile([5, P], bf16)
            nc.sync.dma_start(W1f_t[:], W1f_d[:])
            W2_t = cp.tile([P, P], bf16)
            nc.sync.dma_start(W2_t[:], W2_d[:])
            Wmu_t = cp.tile([P, P], bf16)
            nc.sync.dma_start(Wmu_t[:], Wmu_d[:])
            Wls_t = cp.tile([P, P], bf16)
            nc.sync.dma_start(Wls_t[:], Wls_d[:])
            vecs_t = cp.tile([1, 8 * P], f32)
            nc.sync.dma_start(vecs_t[:], vecs_d[:])
            svecs_t = cp.tile([1, WINS * P], f32)
            nc.sync.dma_start(svecs_t[:], svecs_d[:])


            rrow_bf = cp.tile([1, P], bf16)
            nc.vector.tensor_copy(rrow_bf[:], vecs_t[0:1, 6*P:7*P])
            svec_bf = cp.tile([1, WINS * P], bf16)
            nc.vector.tensor_copy(svec_bf[:], svecs_t[:])

            dma_sems = [nc.alloc_semaphore(f"gsem{i}") for i in range(8)]
            prep_ctr = [0]
            sem_counts = [0] * 8

            # ---------- helpers ----------
            def transform(src_bf, rhs_list):
                kdim = src_bf.shape[-1]
                tps = ps.tile([P, P], bf16, space="PSUM", tag="tpsT")
                nc.tensor.transpose(tps[:kdim, :], src_bf, ident[:])
                tsb = wk.tile([P, P], bf16, tag="tsb")
                nc.vector.tensor_copy(tsb[:kdim, :], tps[:kdim, :])
                outs = []
                for rhs in rhs_list:
                    mps = ps.tile([P, P], f32, space="PSUM", tag="tps")
                    nc.tensor.matmul(mps[:], lhsT=tsb[:kdim, :], rhs=rhs,
                                     start=True, stop=True)
                    outs.append(mps)
                return outs

            pending_bufs = {}

            def emit_preps(tblv, key, b):
                bat = batches[b]
                bufq = []
                for q in range(NQ):
                    ncols = qoff[q][b + 1] - qoff[q][b]
                    buf = gb.tile([P, max(ncols, 1), P], bf16, tag=f"buf{q}")
                    cstart = qoff[q][b]
                    (call,) = bat["qcalls"][q]
                    ncall = len(call)
                    reg = call[0][1]
                    src = tblv[:, reg * P : (reg + 1) * P]
                    nc.gpsimd.dma_gather(
                        buf[:, 0:ncall, :], src,
                        idx_t[q][:, cstart * 8 : (cstart + ncall) * 8],
                        ncall * P, ncall * P, P,
                        elem_step=2 * P,
                        single_packet=False, prepare_only=True,
                        sem=dma_sems[prep_ctr[0] % 8], queue_num=q)
                    sem_counts[prep_ctr[0] % 8] += 1
                    prep_ctr[0] += 1
                    bufq.append(buf)
                pending_bufs[(key, b)] = (bufq, [s for s in sem_counts])

            # ---------- z1 table ----------
            ag_in1 = dr.tile([SLOTS, P], bf16)
            tbl1 = dr.tile([NTBL, P], bf16, addr_space="Shared")
            h_all = sp.tile([5, SLOTS], bf16, tag="hall", bufs=1)
            nc.sync.dma_start(h_all[:], hownT_d[:])
            for b in range(NB):
                zchunk = wk.tile([P, G, P], bf16, tag="zchunk", bufs=2)
                for j in range(G):
                    wi = b * G + j
                    zps = ps.tile([P, P], f32, space="PSUM", tag="tps")
                    nc.tensor.matmul(zps[:], lhsT=h_all[:, wi*P:(wi+1)*P],
                                     rhs=W1f_t[:], start=True, stop=True)
                    nc.vector.tensor_copy(zchunk[:, j, :], zps[:])
                nc.sync.dma_start(
                    ag_in1[b * G * P : (b + 1) * G * P, :].rearrange(
                        "(j p) d -> p j d", p=P),
                    zchunk[:])
            nc.gpsimd.collective_compute(
                "AllGather", AOT.bypass, replica_groups=[list(range(NC))],
                ins=[ag_in1[:]], outs=[tbl1[:]])

            if stage < 2:
                raise StopStage

            # ---------- aggregation ----------
            out_store = st.tile([P, WINS, P], bf16)

            ACTF = mybir.ActivationFunctionType

            def s_build(wi):
                """One tensor_scalar per chunk: S_k = (iota==dl_k)*w_k."""
                cc = int(ccA[wi] + ccB[wi])
                o = win_off[wi]
                s_t = sp.tile([P, CM, P], bf16, tag="s")
                for kk in range(cc):
                    nc.vector.tensor_scalar(
                        s_t[:, kk, :], iota_t[:],
                        meta_t[:, o + kk : o + kk + 1],
                        meta_t[:, TC + o + kk : TC + o + kk + 1],
                        op0=AOT.is_equal, op1=AOT.mult)
                return s_t

            def agg_pass(tbl, layer, key):
                wait_marks = [-1] * 8
                sum_acc = wk.tile([1, P], f32, tag="sacc")
                sq_acc = wk.tile([1, P], f32, tag="qacc")
                nc.gpsimd.memset(sum_acc[:], 0.0)
                nc.gpsimd.memset(sq_acc[:], 0.0)
                tblv = tbl[:, :].rearrange("(a b) d -> a (b d)", b=2)
                for b, bat in enumerate(batches):
                    if (key, b) not in pending_bufs:
                        emit_preps(tblv, key, b)
                    bufq, marks = pending_bufs.pop((key, b))
                    for q in range(NQ):
                        nc.gpsimd.trigger_dma(count=1, queue_num=q)
                    # explicit completion waits on our own gather sems (the
                    # Tile DMASW bridge releases consumers too early on HW)
                    for s in range(8):
                        if marks[s] > wait_marks[s]:
                            nc.tensor.wait_ge(dma_sems[s], 16 * marks[s])
                            wait_marks[s] = marks[s]
                    for wi in range(b * G, (b + 1) * G):
                        cc = int(ccA[wi] + ccB[wi])
                        agg = ps.tile([P, P], f32, space="PSUM", tag="agg",
                                      bufs=3)
                        s_t = s_build(wi)
                        nci = 0
                        ntot = cc + (1 if layer == 1 else 0)
                        for reg, cck, base_k in (
                            (0, int(ccA[wi]), 0),
                            (1, int(ccB[wi]), int(ccA[wi])),
                        ):
                            for k in range(cck):
                                kk = base_k + k
                                q, bb, col = cmap[(wi, reg, k)]
                                assert bb == b
                                nc.tensor.matmul(
                                    agg[:], lhsT=s_t[:, kk, :],
                                    rhs=bufq[q][:, col, :],
                                    start=(nci == 0), stop=(nci == ntot - 1))
                                nci += 1
                        if layer == 1:
                            # correction LAST so no agg-group matmul precedes
                            # the gather deps on the in-order PE queue (the
                            # scheduler would otherwise hoist it before the
                            # z-table transforms the AllGather needs).
                            nc.tensor.matmul(
                                agg[:], lhsT=svec_bf[0:1, wi*P:(wi+1)*P],
                                rhs=rrow_bf[:], start=False, stop=True)
                            nci += 1
                        outw = out_store[:, wi, :]
                        nc.vector.tensor_copy(outw, agg[:])
                        sq = wk.tile([P, P], bf16, tag="sq")
                        nc.scalar.square(sq[:], outw)
                        sps = ps.tile([1, P], f32, space="PSUM", tag="sps",
                                      bufs=1)
                        nc.tensor.matmul(sps[:], lhsT=ones_bf[:], rhs=outw,
                                         start=True, stop=True)
                        nc.vector.tensor_tensor(sum_acc[:], sum_acc[:],
                                                sps[:], op=AOT.add)
                        qps = ps.tile([1, P], f32, space="PSUM", tag="sps",
                                      bufs=1)
                        nc.tensor.matmul(qps[:], lhsT=ones_bf[:], rhs=sq[:],
                                         start=True, stop=True)
                        nc.vector.tensor_tensor(sq_acc[:], sq_acc[:],
                                                qps[:], op=AOT.add)
                return sum_acc, sq_acc

            def bn_reduce(sum_acc, sq_acc, g_row, be_row, name):
                bn_in = dr.tile([1, 2 * P], f32, name=f"bnin_{name}")
                bn_out = dr.tile([1, 2 * P], f32, addr_space="Shared",
                                 name=f"bnout_{name}")
                pack = wk.tile([1, 2 * P], f32, tag="bnpack")
                nc.vector.tensor_copy(pack[0:1, 0:P], sum_acc[:])
                nc.vector.tensor_copy(pack[0:1, P : 2 * P], sq_acc[:])
                nc.sync.dma_start(bn_in[:], pack[:])
                nc.gpsimd.collective_compute(
                    "AllReduce", AOT.add, replica_groups=[list(range(NC))],
                    ins=[bn_in[:]], outs=[bn_out[:]])
                bn_t = wk.tile([1, 2 * P], f32, tag="bnt")
                nc.sync.dma_start(bn_t[:], bn_out[:])
                mean = wk.tile([1, P], f32, tag="bn1")
                nc.vector.tensor_scalar(mean[:], bn_t[0:1, 0:P], 1.0 / N,
                                        None, op0=AOT.mult)
                var = wk.tile([1, P], f32, tag="bn2")
                nc.vector.tensor_scalar(var[:], bn_t[0:1, P : 2 * P], 1.0 / N,
                                        None, op0=AOT.mult)
                msq = wk.tile([1, P], f32, tag="bn3")
                nc.vector.tensor_tensor(msq[:], mean[:], mean[:], op=AOT.mult)
                nc.vector.tensor_tensor(var[:], var[:], msq[:],
                                        op=AOT.subtract)
                nc.vector.tensor_scalar(var[:], var[:], EPS, None, op0=AOT.add)
                rc = wk.tile([1, P], f32, tag="bn3")
                nc.vector.reciprocal(rc[:], var[:])
                rs = wk.tile([1, P], f32, tag="bn3")
                nc.scalar.sqrt(rs[:], rc[:])
                a_row = wk.tile([1, P], f32, tag="bn4")
                nc.vector.tensor_tensor(a_row[:], rs[:], g_row, op=AOT.mult)
                c_row = wk.tile([1, P], f32, tag="bn5")
                nc.vector.tensor_tensor(c_row[:], mean[:], a_row[:],
                                        op=AOT.mult)
                nc.vector.tensor_tensor(c_row[:], be_row, c_row[:],
                                        op=AOT.subtract)
                a_bf = wk.tile([1, P], bf16, tag="bn6")
                nc.vector.tensor_copy(a_bf[:], a_row[:])
                c_bf = wk.tile([1, P], bf16, tag="bn7")
                nc.vector.tensor_copy(c_bf[:], c_row[:])
                af_ps = ps.tile([P, P], f32, space="PSUM", tag="tps")
                nc.tensor.matmul(af_ps[:], lhsT=one_row_bf[:], rhs=a_bf[:],
                                 start=True, stop=True)
                a_full = st.tile([P, P], bf16, name=f"afull_{name}")
                nc.vector.tensor_copy(a_full[:], af_ps[:])
                cf_ps = ps.tile([P, P], f32, space="PSUM", tag="tps")
                nc.tensor.matmul(cf_ps[:], lhsT=one_row_bf[:], rhs=c_bf[:],
                                 start=True, stop=True)
                c_full = st.tile([P, P], bf16, name=f"cfull_{name}")
                nc.vector.tensor_copy(c_full[:], cf_ps[:])
                return a_full, c_full

            _wait_base = [0] * 8

            # ----- layer 1 -----
            sum1, sq1 = agg_pass(tbl1, 1, "L1")
            ag_in2 = dr.tile([SLOTS, P], bf16)
            tbl2 = dr.tile([NTBL, P], bf16, addr_space="Shared")

            a1f, c1f = bn_reduce(sum1, sq1, vecs_t[0:1, 0:P],
                                 vecs_t[0:1, P:2*P], "bn1")
            if debug:
                for wi in range(WINS):
                    o32 = wk.tile([P, P], f32, tag="o32")
                    nc.vector.tensor_copy(o32[:], out_store[:, wi, :])
                    nc.sync.dma_start(dbg_out1_d[wi*P:(wi+1)*P, :], o32[:])
            if stage < 3:
                raise StopStage

            # ----- z2 table -----
            for b in range(NB):
                zchunk = wk.tile([P, G, P], bf16, tag="zchunk", bufs=2)
                for j in range(G):
                    wi = b * G + j
                    x1w = wk.tile([P, P], bf16, tag="x1w")
                    nc.vector.tensor_tensor(x1w[:], out_store[:, wi, :],
                                            a1f[:], op=AOT.mult)
                    nc.vector.tensor_tensor(x1w[:], x1w[:], c1f[:],
                                            op=AOT.add)
                    nc.vector.tensor_scalar(x1w[:], x1w[:], 0.0, None,
                                            op0=AOT.max)
                    (w2ps,) = transform(x1w[:], [W2_t[:]])
                    nc.vector.tensor_copy(zchunk[:, j, :], w2ps[:])
                nc.sync.dma_start(
                    ag_in2[b * G * P : (b + 1) * G * P, :].rearrange(
                        "(j p) d -> p j d", p=P),
                    zchunk[:])
            nc.gpsimd.collective_compute(
                "AllGather", AOT.bypass, replica_groups=[list(range(NC))],
                ins=[ag_in2[:]], outs=[tbl2[:]])

            _wait_base = [sem_counts[s] for s in range(8)]

            # ----- layer 2 -----
            sum2, sq2 = agg_pass(tbl2, 2, "L2")
            a2f, c2f = bn_reduce(sum2, sq2, vecs_t[0:1, 2*P:3*P],
                                 vecs_t[0:1, 3*P:4*P], "bn2")
            if stage < 4:
                raise StopStage

            # ----- heads -----
            bmu_bf = cp.tile([1, P], bf16)
            nc.vector.tensor_copy(bmu_bf[:], vecs_t[0:1, 4*P:5*P])
            bls_bf = cp.tile([1, P], bf16)
            nc.vector.tensor_copy(bls_bf[:], vecs_t[0:1, 5*P:6*P])

            for b in range(NB):
                muc = wk.tile([P, G, P], f32, tag="muc", bufs=2)
                lsc = wk.tile([P, G, P], f32, tag="lsc", bufs=2)
                for j in range(G):
                    wi = b * G + j
                    x2w = wk.tile([P, P], bf16, tag="x1w")
                    nc.vector.tensor_tensor(x2w[:], out_store[:, wi, :],
                                            a2f[:], op=AOT.mult)
                    nc.vector.tensor_tensor(x2w[:], x2w[:], c2f[:],
                                            op=AOT.add)
                    nc.vector.tensor_scalar(x2w[:], x2w[:], 0.0, None,
                                            op0=AOT.max)
                    kdim = P
                    tps = ps.tile([P, P], bf16, space="PSUM", tag="tpsT")
                    nc.tensor.transpose(tps[:], x2w[:], ident[:])
                    tsb = wk.tile([P, P], bf16, tag="tsb")
                    nc.vector.tensor_copy(tsb[:], tps[:])
                    mups = ps.tile([P, P], f32, space="PSUM", tag="tps")
                    nc.tensor.matmul(mups[:], lhsT=tsb[:], rhs=Wmu_t[:],
                                     start=True, stop=False)
                    nc.tensor.matmul(mups[:], lhsT=one_row_bf[:],
                                     rhs=bmu_bf[:], start=False, stop=True)
                    lsps = ps.tile([P, P], f32, space="PSUM", tag="tps")
                    nc.tensor.matmul(lsps[:], lhsT=tsb[:], rhs=Wls_t[:],
                                     start=True, stop=False)
                    nc.tensor.matmul(lsps[:], lhsT=one_row_bf[:],
                                     rhs=bls_bf[:], start=False, stop=True)
                    nc.vector.tensor_copy(muc[:, j, :], mups[:])
                    nc.vector.tensor_copy(lsc[:, j, :], lsps[:])
                nc.sync.dma_start(
                    mu_d[b * G * P : (b + 1) * G * P, :].rearrange(
                        "(j p) d -> p j d", p=P),
                    muc[:])
                nc.sync.dma_start(
                    ls_d[b * G * P : (b + 1) * G * P, :].rearrange(
                        "(j p) d -> p j d", p=P),
                    lsc[:])
          except StopStage:
            pass

    nc.compile()
    return nc


def _plan_sig(plan):
    return (tuple(plan["ccA"]), tuple(plan["ccB"]),
            tuple(plan["qtot"]))


_CACHE = {}


def run(inputs, debug=False, trace=False, stage=5):
    import time
    from concourse.bass_utils import run_bass_kernel_spmd

    t0 = time.time()
    plan, percore = preprocess(inputs["edge_index"], inputs["edge_weight"])
    in_maps = build_in_maps(inputs, plan, percore)
    prep_s = time.time() - t0

    t0 = time.time()
    nc = build_kernel(plan, debug=debug, stage=stage)
    build_s = time.time() - t0

    t0 = time.time()
    res = run_bass_kernel_spmd(nc, in_maps, core_ids=list(range(NC)),
                               trace=trace)
    run_s = time.time() - t0
    print(f"[gcn2] prep {prep_s:.1f}s build {build_s:.1f}s run {run_s:.1f}s",
          flush=True)

    mu = np.zeros((N, P), np.float32)
    ls = np.zeros((N, P), np.float32)
    for c in range(NC):
        if "mu_out" in res.results[c]:
            mu[c * NPC : (c + 1) * NPC] = res.results[c]["mu_out"][:NPC]
            ls[c * NPC : (c + 1) * NPC] = res.results[c]["ls_out"][:NPC]
    return (mu, ls), res


def make_pjrt_runner(nc, in_maps):
    import jax
    from jax.sharding import Mesh, PartitionSpec, NamedSharding
    from jax.experimental.shard_map import shard_map
    from concourse import bass2jax, mybir
    from concourse.bass2jax import _bass_exec_p, install_neuronx_cc_hook

    install_neuronx_cc_hook()
    n_cores = len(in_maps)
    partition_name = nc.partition_id_tensor.name if nc.partition_id_tensor else None
    in_names, out_names, out_avals, zero_outs = [], [], [], []
    for alloc in nc.m.functions[0].allocations:
        if not isinstance(alloc, mybir.MemoryLocationSet):
            continue
        name = alloc.memorylocations[0].name
        if alloc.kind == "ExternalInput":
            if name != partition_name:
                in_names.append(name)
        elif alloc.kind == "ExternalOutput":
            shape = tuple(alloc.tensor_shape)
            dt = mybir.dt.np(alloc.dtype)
            out_avals.append(jax.core.ShapedArray(shape, dt))
            out_names.append(name)
            zero_outs.append(np.zeros(shape, dt))
    n_params = len(in_names)
    n_outs = len(out_avals)
    in_names.extend(out_names)
    if partition_name is not None:
        in_names.append(partition_name)

    def _body(*args):
        operands = list(args)
        if partition_name is not None:
            operands.append(bass2jax.partition_id_tensor())
        outs = _bass_exec_p.bind(
            *operands,
            out_avals=tuple(out_avals), in_names=tuple(in_names),
            out_names=tuple(out_names), lowering_input_output_aliases=(),
            sim_require_finite=True, sim_require_nnan=True, nc=nc)
        return tuple(outs)

    devices = jax.devices()[:n_cores]
    mesh = Mesh(np.asarray(devices), ("core",))
    in_specs = (PartitionSpec("core"),) * (n_params + n_outs)
    out_specs = (PartitionSpec("core"),) * len(out_names)
    sharded = jax.jit(
        shard_map(_body, mesh=mesh, in_specs=in_specs, out_specs=out_specs,
                  check_rep=False),
        keep_unused=True)
    sh = NamedSharding(mesh, PartitionSpec("core"))
    per_core = [[np.asarray(m[name]) for name in in_names[:n_params]]
                for m in in_maps]
    concat_in = [
        jax.device_put(
            np.concatenate([per_core[c][i] for c in range(n_cores)], axis=0),
            sh)
        for i in range(n_params)
    ]
    zeros_dev = [jax.device_put(
                     np.zeros((n_cores * z.shape[0], *z.shape[1:]), z.dtype),
                     sh)
                 for z in zero_outs]

    def execute():
        return sharded(*concat_in, *zeros_dev)

    def unpack(out_arrs):
        return [
            {name: np.asarray(out_arrs[i]).reshape(
                n_cores, *out_avals[i].shape)[c]
             for i, name in enumerate(out_names)}
            for c in range(n_cores)
        ]
    return execute, unpack


def run_timed(inputs, iters=8, stage=5):
    import time, jax
    plan, percore = preprocess(inputs["edge_index"], inputs["edge_weight"])
    in_maps = build_in_maps(inputs, plan, percore)
    key = (_plan_sig(plan), stage)
    if key not in _CACHE:
        _CACHE[key] = build_kernel(plan, stage=stage)
    nc = _CACHE[key]
    execute, unpack = make_pjrt_runner(nc, in_maps)
    t0 = time.time()
    out = execute()
    jax.block_until_ready(out)
    t_first = time.time() - t0
    t0 = time.time()
    last = None
    for _ in range(iters):
        last = execute()
    jax.block_until_ready(last)
    t_total = time.time() - t0
    per_iter_ns = t_total / iters * 1e9
    results = unpack(last)
    mu = np.zeros((N, P), np.float32)
    ls = np.zeros((N, P), np.float32)
    for c in range(NC):
        if "mu_out" in results[c]:
            mu[c * NPC : (c + 1) * NPC] = results[c]["mu_out"][:NPC]
            ls[c * NPC : (c + 1) * NPC] = results[c]["ls_out"][:NPC]
    return (mu, ls), per_iter_ns, t_first


def kernel(**inputs):
    from concourse.bass_utils import run_bass_kernel_spmd

    plan, percore = preprocess(inputs["edge_index"], inputs["edge_weight"])
    in_maps = build_in_maps(inputs, plan, percore)
    key = (_plan_sig(plan), 5)
    if key not in _CACHE:
        _CACHE[key] = build_kernel(plan)
    nc = _CACHE[key]
    res = run_bass_kernel_spmd(nc, in_maps, core_ids=list(range(NC)))

    mu = np.zeros((N, P), np.float32)
    ls = np.zeros((N, P), np.float32)
    for c in range(NC):
        mu[c * NPC : (c + 1) * NPC] = res.results[c]["mu_out"][:NPC]
        ls[c * NPC : (c + 1) * NPC] = res.results[c]["ls_out"][:NPC]
    return (mu, ls)


# revision 13
# speedup vs baseline: 1.9375x; 1.0470x over previous
"""Trainium2 Bass kernel v2: 2-layer GCN encoder on 8 NeuronCores.

Optimizations over v1:
- Host precomputes the symmetric norm per edge (no on-device deg pass,
  no dis scalings).
- BN0 folded into W1 on host; the resulting constant-row term handled
  via a rank-1 (s_dst x r) thin matmul in the aggregation PSUM.
- b1/b2 dropped (BatchNorm right after each conv is shift-invariant).
- bf16 everywhere (2x DVE for the one-hot build, 1-pass PE matmuls).
- dma_gather desc-gen parallelized over 4 SWDGE queues with
  prepare_only + trigger_dma (desc-gen was 65% of runtime).
- Variable per-window chunk counts (uniform across cores = per-window
  max) instead of a global CA/CB max.
"""
import sys

if "/opt/trn_rl_repo" not in sys.path:
    sys.path.insert(0, "/opt/trn_rl_repo")

import numpy as np
import ml_dtypes

BF = ml_dtypes.bfloat16

N = 50000
NC = 8
P = 128
NPC = 6250
WINS = 49
SLOTS = WINS * P        # 6272
NTBL = NC * SLOTS       # 50176
EPS = 1e-5
A_HI = 32768
B_LO = NTBL - 32768     # 17408
G = 7                   # windows per gather batch
NB = WINS // G
NQ = 4                  # SWDGE queues


def q_of(i):
    return (i // NPC) * SLOTS + (i % NPC)


def wrap_idx16(idx_flat):
    n = len(idx_flat)
    assert n % 16 == 0
    base = idx_flat.reshape(n // 16, 16).T.astype(np.int16)
    return np.tile(base, (8, 1))


def preprocess(edge_index, edge_weight):
    """Chunk plan (uniform across cores) + per-core tensors."""
    src = np.asarray(edge_index[0], np.int64)
    dst = np.asarray(edge_index[1], np.int64)
    w = np.asarray(edge_weight, np.float32)
    loop = np.arange(N, dtype=np.int64)
    src = np.concatenate([src, loop])
    dst = np.concatenate([dst, loop])
    w = np.concatenate([w, np.ones(N, np.float32)])

    deg = np.bincount(dst, weights=w.astype(np.float64), minlength=N)
    dis = np.where(deg > 0, 1.0 / np.sqrt(np.maximum(deg, 1e-12)), 0.0)
    norm = (dis[src] * w * dis[dst]).astype(np.float32)

    qsrc = q_of(src)
    core = dst // NPC
    wloc = (dst % NPC) // P
    dloc = (dst % NPC) % P

    svec_all = np.zeros((NC, WINS, P), np.float32)
    np.add.at(svec_all, (core, wloc, dloc), norm)

    percore_raw = []
    nA = np.zeros((NC, WINS), np.int64)
    nB = np.zeros((NC, WINS), np.int64)
    for c in range(NC):
        m = core == c
        qs_c, dl_c, nm_c, wl_c = qsrc[m], dloc[m], norm[m], wloc[m]
        isA = (qs_c % 2) == 0          # parity region: even rows
        o = np.lexsort((qs_c, ~isA, wl_c))
        qs_c, dl_c, nm_c, wl_c, isA = (
            qs_c[o], dl_c[o], nm_c[o], wl_c[o], isA[o])
        percore_raw.append((qs_c, dl_c, nm_c, wl_c, isA))
        np.add.at(nA[c], wl_c[isA], 1)
        np.add.at(nB[c], wl_c[~isA], 1)

    ccA = np.maximum(1, -(-nA.max(axis=0) // P)).astype(int)   # [WINS]
    ccB = np.maximum(1, -(-nB.max(axis=0) // P)).astype(int)

    # global chunk list, window-major; key (wi, reg, k) with region-local k
    chunk_list = []
    for wi in range(WINS):
        for k in range(ccA[wi]):
            chunk_list.append((wi, 0, k))
        for k in range(ccB[wi]):
            chunk_list.append((wi, 1, k))
    TC = len(chunk_list)

    # cumulative chunk offsets per window
    win_off = {}
    off = 0
    for wi in range(WINS):
        win_off[wi] = off
        off += ccA[wi] + ccB[wi]
    assert off == TC

    # batches: split each batch's chunks into NQ near-equal contiguous
    # groups; each group -> 1-2 calls (per contiguous region run)
    batches = []
    cmap = {}
    qoff = [[0] * (NB + 1) for _ in range(NQ)]   # chunk offset per queue
    for b in range(NB):
        wlo, whi = b * G, (b + 1) * G
        bchunks = sorted((t for t in chunk_list if wlo <= t[0] < whi),
                         key=lambda t: (t[1], t[0], t[2]))
        echunks = [t for t in bchunks if t[1] == 0]
        ochunks = [t for t in bchunks if t[1] == 1]
        ne, no = len(echunks), len(ochunks)
        groups = [echunks[: (ne + 1) // 2], echunks[(ne + 1) // 2 :],
                  ochunks[: (no + 1) // 2], ochunks[(no + 1) // 2 :]]
        qcalls = []
        for q in range(NQ):
            assert groups[q], f"empty queue group b={b} q={q}"
            qcalls.append([groups[q]])
            col = 0
            for t in groups[q]:
                cmap[t] = (q, b, col)
                col += 1
            qoff[q][b + 1] = qoff[q][b] + col
        batches.append(dict(qcalls=qcalls))

    plan = dict(ccA=ccA, ccB=ccB, TC=TC, batches=batches, cmap=cmap,
                win_off=win_off, qoff=qoff,
                qtot=[qoff[q][NB] for q in range(NQ)])

    percore = []
    for c in range(NC):
        qs_c, dl_c, nm_c, wl_c, isA_c = percore_raw[c]
        idx_by = {}
        for wi in range(WINS):
            mw = wl_c == wi
            for reg in (0, 1):
                mr = mw & (isA_c if reg == 0 else ~isA_c)
                idx_by[(wi, reg)] = (qs_c[mr], dl_c[mr], nm_c[mr])

        # meta: [all dl cols | all w cols], f32 (ACT scale requires FP32)
        meta = np.zeros((P, 2 * TC), np.float32)
        qidx = [[] for _ in range(NQ)]   # (b, col, idx_vec[128])
        for wi in range(WINS):
            cc = ccA[wi] + ccB[wi]
            o = win_off[wi]
            for reg, cck, base_k in ((0, ccA[wi], 0), (1, ccB[wi], ccA[wi])):
                qs, dl, nm = idx_by[(wi, reg)]
                for k in range(cck):
                    lo, hi = k * P, min((k + 1) * P, len(qs))
                    nk = max(0, hi - lo)
                    kk = base_k + k
                    if nk > 0:
                        meta[:nk, o + kk] = dl[lo:hi]
                        meta[:nk, TC + o + kk] = nm[lo:hi]
                    iv = np.zeros(P, np.int64)
                    if nk > 0:
                        iv[:nk] = qs[lo:hi] // 2
                    q, b, col = cmap[(wi, reg, k)]
                    qidx[q].append((b, col, iv))

        qidx16 = []
        for q in range(NQ):
            qidx[q].sort(key=lambda t: (t[0], t[1]))
            assert len(qidx[q]) == plan["qtot"][q]
            flat = np.concatenate([t[2] for t in qidx[q]])
            qidx16.append(wrap_idx16(flat))

        percore.append(dict(meta=meta, qidx16=qidx16,
                            svec=svec_all[c].reshape(1, WINS * P)))
    # batch chunk offsets (global chunk order is window-major => contiguous
    # per batch)
    boff = [0] * (NB + 1)
    t = 0
    for b in range(NB):
        nb_ = sum(ccA[wi] + ccB[wi] for wi in range(b * G, (b + 1) * G))
        boff[b + 1] = boff[b] + nb_
    plan["boff"] = boff
    return plan, percore


def build_in_maps(inputs, plan, percore):
    h = np.asarray(inputs["h"], np.float32)
    m0 = h.mean(axis=0)
    v0 = h.var(axis=0)
    a0 = np.asarray(inputs["g0"], np.float32) / np.sqrt(v0 + EPS)
    c0 = np.asarray(inputs["be0"], np.float32) - m0 * a0
    W1 = np.asarray(inputs["W1"], np.float32)
    W1f = (a0[:, None] * W1).astype(BF)
    rrow = (c0 @ W1).astype(np.float32)

    vecs = np.zeros((1, 8 * P), np.float32)
    vecs[0, 0*P:1*P] = np.asarray(inputs["g1"], np.float32)
    vecs[0, 1*P:2*P] = np.asarray(inputs["be1"], np.float32)
    vecs[0, 2*P:3*P] = np.asarray(inputs["g2"], np.float32)
    vecs[0, 3*P:4*P] = np.asarray(inputs["be2"], np.float32)
    vecs[0, 4*P:5*P] = np.asarray(inputs["bmu"], np.float32)
    vecs[0, 5*P:6*P] = np.asarray(inputs["bls"], np.float32)
    vecs[0, 6*P:7*P] = rrow

    W2 = np.asarray(inputs["W2"], np.float32).astype(BF)
    Wmu = np.asarray(inputs["Wmu"], np.float32).astype(BF)
    Wls = np.asarray(inputs["Wls"], np.float32).astype(BF)

    in_maps = []
    for c in range(NC):
        d = percore[c]
        hown = np.zeros((SLOTS, 5), np.float32)
        hown[:NPC] = h[c * NPC : (c + 1) * NPC]
        in_maps.append({
            "hownT": np.ascontiguousarray(hown.T).astype(BF),
            "meta": d["meta"],
            "idxQ0": d["qidx16"][0],
            "idxQ1": d["qidx16"][1],
            "idxQ2": d["qidx16"][2],
            "idxQ3": d["qidx16"][3],
            "W1f": W1f,
            "W2": W2,
            "Wmu": Wmu,
            "Wls": Wls,
            "vecs": vecs,
            "svecs": d["svec"],
        })
    return in_maps


def build_kernel(plan, debug=False, stage=5):
    import concourse.bacc as bacc
    import concourse.tile as tile
    from concourse import mybir
    from concourse.masks import make_identity

    f32 = mybir.dt.float32
    bf16 = mybir.dt.bfloat16
    i16 = mybir.dt.int16
    AOT = mybir.AluOpType

    ccA, ccB = plan["ccA"], plan["ccB"]
    TC = plan["TC"]
    batches = plan["batches"]
    cmap = plan["cmap"]
    win_off = plan["win_off"]
    qoff = plan["qoff"]
    qtot = plan["qtot"]
    CM = int(max(ccA[wi] + ccB[wi] for wi in range(WINS)))

    nc = bacc.Bacc("TRN2", num_devices=NC, num_swdge_queues=NQ)

    hownT_d = nc.dram_tensor("hownT", [5, SLOTS], bf16, kind="ExternalInput")
    meta_d = nc.dram_tensor("meta", [P, 2 * TC], f32, kind="ExternalInput")
    idx_d = [nc.dram_tensor(f"idxQ{q}", [P, qtot[q] * 8], i16,
                            kind="ExternalInput") for q in range(NQ)]
    W1f_d = nc.dram_tensor("W1f", [5, P], bf16, kind="ExternalInput")
    W2_d = nc.dram_tensor("W2", [P, P], bf16, kind="ExternalInput")
    Wmu_d = nc.dram_tensor("Wmu", [P, P], bf16, kind="ExternalInput")
    Wls_d = nc.dram_tensor("Wls", [P, P], bf16, kind="ExternalInput")
    vecs_d = nc.dram_tensor("vecs", [1, 8 * P], f32, kind="ExternalInput")
    svecs_d = nc.dram_tensor("svecs", [1, WINS * P], f32, kind="ExternalInput")
    mu_d = nc.dram_tensor("mu_out", [SLOTS, P], f32, kind="ExternalOutput")
    ls_d = nc.dram_tensor("ls_out", [SLOTS, P], f32, kind="ExternalOutput")
    if debug:
        dbg_out1_d = nc.dram_tensor("dbg_out1", [SLOTS, P], f32,
                                    kind="ExternalOutput")

    class StopStage(Exception):
        pass

    with tile.TileContext(nc) as tc:
        with (
            tc.tile_pool(name="const", bufs=1) as cp,
            tc.tile_pool(name="store", bufs=1) as st,
            tc.tile_pool(name="work", bufs=3) as wk,
            tc.tile_pool(name="spool", bufs=3) as sp,
            tc.tile_pool(name="gbuf", bufs=2) as gb,
            tc.tile_pool(name="psum", bufs=2, space="PSUM") as ps,
            tc.tile_pool(name="dram", bufs=1, space="DRAM") as dr,
        ):
          try:
            # ---------- constants ----------
            iota_t = cp.tile([P, P], f32)
            nc.gpsimd.iota(iota_t[:], pattern=[[1, P]], base=0,
                           channel_multiplier=0,
                           allow_small_or_imprecise_dtypes=True)
            ident = cp.tile([P, P], bf16)
            make_identity(nc, ident[:])
            ones_bf = cp.tile([P, 1], bf16)
            nc.gpsimd.memset(ones_bf[:], 1.0)
            one_row = cp.tile([1, P], f32)
            nc.gpsimd.memset(one_row[:], 1.0)
            one_row_bf = cp.tile([1, P], bf16)
            nc.gpsimd.memset(one_row_bf[:], 1.0)

            meta_t = cp.tile([P, 2 * TC], f32)
            nc.sync.dma_start(meta_t[:], meta_d[:])
            idx_t = []
            for q in range(NQ):
                t = cp.tile([P, qtot[q] * 8], i16, name=f"idxt{q}")
                nc.sync.dma_start(t[:], idx_d[q][:])
                idx_t.append(t)
            W1f_t = cp.tile([5, P], bf16)
            nc.sync.dma_start(W1f_t[:], W1f_d[:])
            W2_t = cp.tile([P, P], bf16)
            nc.sync.dma_start(W2_t[:], W2_d[:])
            Wmu_t = cp.tile([P, P], bf16)
            nc.sync.dma_start(Wmu_t[:], Wmu_d[:])
            Wls_t = cp.tile([P, P], bf16)
            nc.sync.dma_start(Wls_t[:], Wls_d[:])
            vecs_t = cp.tile([1, 8 * P], f32)
            nc.sync.dma_start(vecs_t[:], vecs_d[:])
            svecs_t = cp.tile([1, WINS * P], f32)
            nc.sync.dma_start(svecs_t[:], svecs_d[:])


            rrow_bf = cp.tile([1, P], bf16)
            nc.vector.tensor_copy(rrow_bf[:], vecs_t[0:1, 6*P:7*P])
            svec_bf = cp.tile([1, WINS * P], bf16)
            nc.vector.tensor_copy(svec_bf[:], svecs_t[:])

            dma_sems = [nc.alloc_semaphore(f"gsem{i}") for i in range(8)]
            prep_ctr = [0]
            sem_counts = [0] * 8

            # ---------- helpers ----------
            def transform(src_bf, rhs_list):
                kdim = src_bf.shape[-1]
                tps = ps.tile([P, P], bf16, space="PSUM", tag="tpsT")
                nc.tensor.transpose(tps[:kdim, :], src_bf, ident[:])
                tsb = wk.tile([P, P], bf16, tag="tsb")
                nc.vector.tensor_copy(tsb[:kdim, :], tps[:kdim, :])
                outs = []
                for rhs in rhs_list:
                    mps = ps.tile([P, P], f32, space="PSUM", tag="tps")
                    nc.tensor.matmul(mps[:], lhsT=tsb[:kdim, :], rhs=rhs,
                                     start=True, stop=True)
                    outs.append(mps)
                return outs

            pending_bufs = {}

            def emit_preps(tblv, key, b):
                bat = batches[b]
                bufq = []
                for q in range(NQ):
                    ncols = qoff[q][b + 1] - qoff[q][b]
                    buf = gb.tile([P, max(ncols, 1), P], bf16, tag=f"buf{q}")
                    cstart = qoff[q][b]
                    (call,) = bat["qcalls"][q]
                    ncall = len(call)
                    reg = call[0][1]
                    src = tblv[:, reg * P : (reg + 1) * P]
                    nc.gpsimd.dma_gather(
                        buf[:, 0:ncall, :], src,
                        idx_t[q][:, cstart * 8 : (cstart + ncall) * 8],
                        ncall * P, ncall * P, P,
                        elem_step=2 * P,
                        single_packet=False, prepare_only=True,
                        sem=dma_sems[prep_ctr[0] % 8], queue_num=q)
                    sem_counts[prep_ctr[0] % 8] += 1
                    prep_ctr[0] += 1
                    bufq.append(buf)
                pending_bufs[(key, b)] = (bufq, [s for s in sem_counts])

            # ---------- z1 table ----------
            ag_in1 = dr.tile([SLOTS, P], bf16)
            tbl1 = dr.tile([NTBL, P], bf16, addr_space="Shared")
            h_all = sp.tile([5, SLOTS], bf16, tag="sst", bufs=2)
            nc.sync.dma_start(h_all[:], hownT_d[:])
            for b in range(NB):
                zchunk = wk.tile([P, G, P], bf16, tag="zchunk", bufs=2)
                for j in range(G):
                    wi = b * G + j
                    zps = ps.tile([P, P], f32, space="PSUM", tag="tps")
                    nc.tensor.matmul(zps[:], lhsT=h_all[:, wi*P:(wi+1)*P],
                                     rhs=W1f_t[:], start=True, stop=True)
                    nc.vector.tensor_copy(zchunk[:, j, :], zps[:])
                nc.sync.dma_start(
                    ag_in1[b * G * P : (b + 1) * G * P, :].rearrange(
                        "(j p) d -> p j d", p=P),
                    zchunk[:])
            nc.gpsimd.collective_compute(
                "AllGather", AOT.bypass, replica_groups=[list(range(NC))],
                ins=[ag_in1[:]], outs=[tbl1[:]])

            if stage < 2:
                raise StopStage

            # ---------- aggregation ----------
            out_store = st.tile([P, WINS, P], bf16)

            ACTF = mybir.ActivationFunctionType

            # scatter matrices built once (layer 1), spilled to DRAM, and
            # streamed back per batch in layer 2 (saves ~440us of DVE)
            S_dram = dr.tile([P, TC * P], bf16)
            # half-batch S streaming in layer 2 (SBUF pressure)
            halves = []
            for b in range(NB):
                for wlo, whi in ((b * G, b * G + 4), (b * G + 4, (b + 1) * G)):
                    c0 = win_off[wlo]
                    c1 = (win_off[whi] if whi < WINS else TC)
                    halves.append((wlo, whi, c0, c1))
            HMAX = max(c1 - c0 for _, _, c0, c1 in halves)
            half_of = {}
            for hidx, (wlo, whi, c0, c1) in enumerate(halves):
                for wi in range(wlo, whi):
                    half_of[wi] = (hidx, c0, c1)

            def s_build(wi):
                """One tensor_scalar per chunk: S_k = (iota==dl_k)*w_k."""
                cc = int(ccA[wi] + ccB[wi])
                o = win_off[wi]
                s_t = sp.tile([P, CM, P], bf16, tag="s", bufs=2)
                for kk in range(cc):
                    nc.vector.tensor_scalar(
                        s_t[:, kk, :], iota_t[:],
                        meta_t[:, o + kk : o + kk + 1],
                        meta_t[:, TC + o + kk : TC + o + kk + 1],
                        op0=AOT.is_equal, op1=AOT.mult)
                nc.sync.dma_start(
                    S_dram[:, o * P : (o + cc) * P].rearrange(
                        "p (t x) -> p t x", x=P),
                    s_t[:, :cc, :])
                return s_t

            def agg_pass(tbl, layer, key):
                wait_marks = [-1] * 8
                sum_acc = wk.tile([1, P], f32, tag="sacc")
                sq_acc = wk.tile([1, P], f32, tag="qacc")
                nc.gpsimd.memset(sum_acc[:], 0.0)
                nc.gpsimd.memset(sq_acc[:], 0.0)
                tblv = tbl[:, :].rearrange("(a b) d -> a (b d)", b=2)
                cur_half = [-1]

                def load_half(wi):
                    hidx, c0, c1 = half_of[wi]
                    if hidx == cur_half[0]:
                        return load_half.s_bt, c0
                    s_bt = sp.tile([P, HMAX, P], bf16, tag="sst", bufs=2)
                    nc.sync.dma_start(
                        s_bt[:, : c1 - c0, :],
                        S_dram[:, c0 * P : c1 * P].rearrange(
                            "p (t x) -> p t x", x=P))
                    cur_half[0] = hidx
                    load_half.s_bt = s_bt
                    return s_bt, c0

                for b, bat in enumerate(batches):
                    if (key, b) not in pending_bufs:
                        emit_preps(tblv, key, b)
                    bufq, marks = pending_bufs.pop((key, b))
                    for q in range(NQ):
                        nc.gpsimd.trigger_dma(count=1, queue_num=q)
                    # explicit completion waits on our own gather sems (the
                    # Tile DMASW bridge releases consumers too early on HW)
                    for s in range(8):
                        if marks[s] > wait_marks[s]:
                            nc.tensor.wait_ge(dma_sems[s], 16 * marks[s])
                            wait_marks[s] = marks[s]
                    for wi in range(b * G, (b + 1) * G):
                        cc = int(ccA[wi] + ccB[wi])
                        agg = ps.tile([P, P], f32, space="PSUM", tag="agg",
                                      bufs=3)
                        if layer == 1:
                            s_t = s_build(wi)
                            s_off = 0
                        else:
                            s_t, c0 = load_half(wi)
                            s_off = win_off[wi] - c0
                        nci = 0
                        ntot = cc + (1 if layer == 1 else 0)
                        for reg, cck, base_k in (
                            (0, int(ccA[wi]), 0),
                            (1, int(ccB[wi]), int(ccA[wi])),
                        ):
                            for k in range(cck):
                                kk = base_k + k
                                q, bb, col = cmap[(wi, reg, k)]
                                assert bb == b
                                nc.tensor.matmul(
                                    agg[:], lhsT=s_t[:, s_off + kk, :],
                                    rhs=bufq[q][:, col, :],
                                    start=(nci == 0), stop=(nci == ntot - 1))
                                nci += 1
                        if layer == 1:
                            # correction LAST so no agg-group matmul precedes
                            # the gather deps on the in-order PE queue (the
                            # scheduler would otherwise hoist it before the
                            # z-table transforms the AllGather needs).
                            nc.tensor.matmul(
                                agg[:], lhsT=svec_bf[0:1, wi*P:(wi+1)*P],
                                rhs=rrow_bf[:], start=False, stop=True)
                            nci += 1
                        outw = out_store[:, wi, :]
                        nc.vector.tensor_copy(outw, agg[:])
                        sq = wk.tile([P, P], bf16, tag="sq")
                        nc.scalar.square(sq[:], outw)
                        sps = ps.tile([1, P], f32, space="PSUM", tag="sps",
                                      bufs=1)
                        nc.tensor.matmul(sps[:], lhsT=ones_bf[:], rhs=outw,
                                         start=True, stop=True)
                        nc.vector.tensor_tensor(sum_acc[:], sum_acc[:],
                                                sps[:], op=AOT.add)
                        qps = ps.tile([1, P], f32, space="PSUM", tag="sps",
                                      bufs=1)
                        nc.tensor.matmul(qps[:], lhsT=ones_bf[:], rhs=sq[:],
                                         start=True, stop=True)
                        nc.vector.tensor_tensor(sq_acc[:], sq_acc[:],
                                                qps[:], op=AOT.add)
                return sum_acc, sq_acc

            def bn_reduce(sum_acc, sq_acc, g_row, be_row, name):
                bn_in = dr.tile([1, 2 * P], f32, name=f"bnin_{name}")
                bn_out = dr.tile([1, 2 * P], f32, addr_space="Shared",
                                 name=f"bnout_{name}")
                pack = wk.tile([1, 2 * P], f32, tag="bnpack")
                nc.vector.tensor_copy(pack[0:1, 0:P], sum_acc[:])
                nc.vector.tensor_copy(pack[0:1, P : 2 * P], sq_acc[:])
                nc.sync.dma_start(bn_in[:], pack[:])
                nc.gpsimd.collective_compute(
                    "AllReduce", AOT.add, replica_groups=[list(range(NC))],
                    ins=[bn_in[:]], outs=[bn_out[:]])
                bn_t = wk.tile([1, 2 * P], f32, tag="bnt")
                nc.sync.dma_start(bn_t[:], bn_out[:])
                mean = wk.tile([1, P], f32, tag="bn1")
                nc.vector.tensor_scalar(mean[:], bn_t[0:1, 0:P], 1.0 / N,
                                        None, op0=AOT.mult)
                var = wk.tile([1, P], f32, tag="bn2")
                nc.vector.tensor_scalar(var[:], bn_t[0:1, P : 2 * P], 1.0 / N,
                                        None, op0=AOT.mult)
                msq = wk.tile([1, P], f32, tag="bn3")
                nc.vector.tensor_tensor(msq[:], mean[:], mean[:], op=AOT.mult)
                nc.vector.tensor_tensor(var[:], var[:], msq[:],
                                        op=AOT.subtract)
                nc.vector.tensor_scalar(var[:], var[:], EPS, None, op0=AOT.add)
                rc = wk.tile([1, P], f32, tag="bn3")
                nc.vector.reciprocal(rc[:], var[:])
                rs = wk.tile([1, P], f32, tag="bn3")
                nc.scalar.sqrt(rs[:], rc[:])
                a_row = wk.tile([1, P], f32, tag="bn4")
                nc.vector.tensor_tensor(a_row[:], rs[:], g_row, op=AOT.mult)
                c_row = wk.tile([1, P], f32, tag="bn5")
                nc.vector.tensor_tensor(c_row[:], mean[:], a_row[:],
                                        op=AOT.mult)
                nc.vector.tensor_tensor(c_row[:], be_row, c_row[:],
                                        op=AOT.subtract)
                a_bf = wk.tile([1, P], bf16, tag="bn6")
                nc.vector.tensor_copy(a_bf[:], a_row[:])
                c_bf = wk.tile([1, P], bf16, tag="bn7")
                nc.vector.tensor_copy(c_bf[:], c_row[:])
                af_ps = ps.tile([P, P], f32, space="PSUM", tag="tps")
                nc.tensor.matmul(af_ps[:], lhsT=one_row_bf[:], rhs=a_bf[:],
                                 start=True, stop=True)
                a_full = st.tile([P, P], bf16, name=f"afull_{name}")
                nc.vector.tensor_copy(a_full[:], af_ps[:])
                cf_ps = ps.tile([P, P], f32, space="PSUM", tag="tps")
                nc.tensor.matmul(cf_ps[:], lhsT=one_row_bf[:], rhs=c_bf[:],
                                 start=True, stop=True)
                c_full = st.tile([P, P], bf16, name=f"cfull_{name}")
                nc.vector.tensor_copy(c_full[:], cf_ps[:])
                return a_full, c_full

            _wait_base = [0] * 8

            # ----- layer 1 -----
            sum1, sq1 = agg_pass(tbl1, 1, "L1")
            ag_in2 = dr.tile([SLOTS, P], bf16)
            tbl2 = dr.tile([NTBL, P], bf16, addr_space="Shared")

            a1f, c1f = bn_reduce(sum1, sq1, vecs_t[0:1, 0:P],
                                 vecs_t[0:1, P:2*P], "bn1")
            if debug:
                for wi in range(WINS):
                    o32 = wk.tile([P, P], f32, tag="o32")
                    nc.vector.tensor_copy(o32[:], out_store[:, wi, :])
                    nc.sync.dma_start(dbg_out1_d[wi*P:(wi+1)*P, :], o32[:])
            if stage < 3:
                raise StopStage

            # ----- z2 table -----
            for b in range(NB):
                zchunk = wk.tile([P, G, P], bf16, tag="zchunk", bufs=2)
                for j in range(G):
                    wi = b * G + j
                    x1w = wk.tile([P, P], bf16, tag="x1w")
                    nc.vector.tensor_tensor(x1w[:], out_store[:, wi, :],
                                            a1f[:], op=AOT.mult)
                    nc.vector.tensor_tensor(x1w[:], x1w[:], c1f[:],
                                            op=AOT.add)
                    nc.vector.tensor_scalar(x1w[:], x1w[:], 0.0, None,
                                            op0=AOT.max)
                    (w2ps,) = transform(x1w[:], [W2_t[:]])
                    nc.vector.tensor_copy(zchunk[:, j, :], w2ps[:])
                nc.sync.dma_start(
                    ag_in2[b * G * P : (b + 1) * G * P, :].rearrange(
                        "(j p) d -> p j d", p=P),
                    zchunk[:])
            nc.gpsimd.collective_compute(
                "AllGather", AOT.bypass, replica_groups=[list(range(NC))],
                ins=[ag_in2[:]], outs=[tbl2[:]])

            _wait_base = [sem_counts[s] for s in range(8)]

            # ----- layer 2 -----
            sum2, sq2 = agg_pass(tbl2, 2, "L2")
            a2f, c2f = bn_reduce(sum2, sq2, vecs_t[0:1, 2*P:3*P],
                                 vecs_t[0:1, 3*P:4*P], "bn2")
            if stage < 4:
                raise StopStage

            # ----- heads -----
            bmu_bf = cp.tile([1, P], bf16)
            nc.vector.tensor_copy(bmu_bf[:], vecs_t[0:1, 4*P:5*P])
            bls_bf = cp.tile([1, P], bf16)
            nc.vector.tensor_copy(bls_bf[:], vecs_t[0:1, 5*P:6*P])

            for b in range(NB):
                muc = wk.tile([P, G, P], f32, tag="muc", bufs=2)
                lsc = wk.tile([P, G, P], f32, tag="lsc", bufs=2)
                for j in range(G):
                    wi = b * G + j
                    x2w = wk.tile([P, P], bf16, tag="x1w")
                    nc.vector.tensor_tensor(x2w[:], out_store[:, wi, :],
                                            a2f[:], op=AOT.mult)
                    nc.vector.tensor_tensor(x2w[:], x2w[:], c2f[:],
                                            op=AOT.add)
                    nc.vector.tensor_scalar(x2w[:], x2w[:], 0.0, None,
                                            op0=AOT.max)
                    kdim = P
                    tps = ps.tile([P, P], bf16, space="PSUM", tag="tpsT")
                    nc.tensor.transpose(tps[:], x2w[:], ident[:])
                    tsb = wk.tile([P, P], bf16, tag="tsb")
                    nc.vector.tensor_copy(tsb[:], tps[:])
                    mups = ps.tile([P, P], f32, space="PSUM", tag="tps")
                    nc.tensor.matmul(mups[:], lhsT=tsb[:], rhs=Wmu_t[:],
                                     start=True, stop=False)
                    nc.tensor.matmul(mups[:], lhsT=one_row_bf[:],
                                     rhs=bmu_bf[:], start=False, stop=True)
                    lsps = ps.tile([P, P], f32, space="PSUM", tag="tps")
                    nc.tensor.matmul(lsps[:], lhsT=tsb[:], rhs=Wls_t[:],
                                     start=True, stop=False)
                    nc.tensor.matmul(lsps[:], lhsT=one_row_bf[:],
                                     rhs=bls_bf[:], start=False, stop=True)
                    nc.vector.tensor_copy(muc[:, j, :], mups[:])
                    nc.vector.tensor_copy(lsc[:, j, :], lsps[:])
                nc.sync.dma_start(
                    mu_d[b * G * P : (b + 1) * G * P, :].rearrange(
                        "(j p) d -> p j d", p=P),
                    muc[:])
                nc.sync.dma_start(
                    ls_d[b * G * P : (b + 1) * G * P, :].rearrange(
                        "(j p) d -> p j d", p=P),
                    lsc[:])
          except StopStage:
            pass

    nc.compile()
    return nc


def _plan_sig(plan):
    return (tuple(plan["ccA"]), tuple(plan["ccB"]),
            tuple(plan["qtot"]))


_CACHE = {}


def run(inputs, debug=False, trace=False, stage=5):
    import time
    from concourse.bass_utils import run_bass_kernel_spmd

    t0 = time.time()
    plan, percore = preprocess(inputs["edge_index"], inputs["edge_weight"])
    in_maps = build_in_maps(inputs, plan, percore)
    prep_s = time.time() - t0

    t0 = time.time()
    nc = build_kernel(plan, debug=debug, stage=stage)
    build_s = time.time() - t0

    t0 = time.time()
    res = run_bass_kernel_spmd(nc, in_maps, core_ids=list(range(NC)),
                               trace=trace)
    run_s = time.time() - t0
    print(f"[gcn2] prep {prep_s:.1f}s build {build_s:.1f}s run {run_s:.1f}s",
          flush=True)

    mu = np.zeros((N, P), np.float32)
    ls = np.zeros((N, P), np.float32)
    for c in range(NC):
        if "mu_out" in res.results[c]:
            mu[c * NPC : (c + 1) * NPC] = res.results[c]["mu_out"][:NPC]
            ls[c * NPC : (c + 1) * NPC] = res.results[c]["ls_out"][:NPC]
    return (mu, ls), res


def make_pjrt_runner(nc, in_maps):
    import jax
    from jax.sharding import Mesh, PartitionSpec, NamedSharding
    from jax.experimental.shard_map import shard_map
    from concourse import bass2jax, mybir
    from concourse.bass2jax import _bass_exec_p, install_neuronx_cc_hook

    install_neuronx_cc_hook()
    n_cores = len(in_maps)
    partition_name = nc.partition_id_tensor.name if nc.partition_id_tensor else None
    in_names, out_names, out_avals, zero_outs = [], [], [], []
    for alloc in nc.m.functions[0].allocations:
        if not isinstance(alloc, mybir.MemoryLocationSet):
            continue
        name = alloc.memorylocations[0].name
        if alloc.kind == "ExternalInput":
            if name != partition_name:
                in_names.append(name)
        elif alloc.kind == "ExternalOutput":
            shape = tuple(alloc.tensor_shape)
            dt = mybir.dt.np(alloc.dtype)
            out_avals.append(jax.core.ShapedArray(shape, dt))
            out_names.append(name)
            zero_outs.append(np.zeros(shape, dt))
    n_params = len(in_names)
    n_outs = len(out_avals)
    in_names.extend(out_names)
    if partition_name is not None:
        in_names.append(partition_name)

    def _body(*args):
        operands = list(args)
        if partition_name is not None:
            operands.append(bass2jax.partition_id_tensor())
        outs = _bass_exec_p.bind(
            *operands,
            out_avals=tuple(out_avals), in_names=tuple(in_names),
            out_names=tuple(out_names), lowering_input_output_aliases=(),
            sim_require_finite=True, sim_require_nnan=True, nc=nc)
        return tuple(outs)

    devices = jax.devices()[:n_cores]
    mesh = Mesh(np.asarray(devices), ("core",))
    in_specs = (PartitionSpec("core"),) * (n_params + n_outs)
    out_specs = (PartitionSpec("core"),) * len(out_names)
    sharded = jax.jit(
        shard_map(_body, mesh=mesh, in_specs=in_specs, out_specs=out_specs,
                  check_rep=False),
        keep_unused=True)
    sh = NamedSharding(mesh, PartitionSpec("core"))
    per_core = [[np.asarray(m[name]) for name in in_names[:n_params]]
                for m in in_maps]
    concat_in = [
        jax.device_put(
            np.concatenate([per_core[c][i] for c in range(n_cores)], axis=0),
            sh)
        for i in range(n_params)
    ]
    zeros_dev = [jax.device_put(
                     np.zeros((n_cores * z.shape[0], *z.shape[1:]), z.dtype),
                     sh)
                 for z in zero_outs]

    def execute():
        return sharded(*concat_in, *zeros_dev)

    def unpack(out_arrs):
        return [
            {name: np.asarray(out_arrs[i]).reshape(
                n_cores, *out_avals[i].shape)[c]
             for i, name in enumerate(out_names)}
            for c in range(n_cores)
        ]
    return execute, unpack


def run_timed(inputs, iters=8, stage=5):
    import time, jax
    plan, percore = preprocess(inputs["edge_index"], inputs["edge_weight"])
    in_maps = build_in_maps(inputs, plan, percore)
    key = (_plan_sig(plan), stage)
    if key not in _CACHE:
        _CACHE[key] = build_kernel(plan, stage=stage)
    nc = _CACHE[key]
    execute, unpack = make_pjrt_runner(nc, in_maps)
    t0 = time.time()
    out = execute()
    jax.block_until_ready(out)
    t_first = time.time() - t0
    t0 = time.time()
    last = None
    for _ in range(iters):
        last = execute()
    jax.block_until_ready(last)
    t_total = time.time() - t0
    per_iter_ns = t_total / iters * 1e9
    results = unpack(last)
    mu = np.zeros((N, P), np.float32)
    ls = np.zeros((N, P), np.float32)
    for c in range(NC):
        if "mu_out" in results[c]:
            mu[c * NPC : (c + 1) * NPC] = results[c]["mu_out"][:NPC]
            ls[c * NPC : (c + 1) * NPC] = results[c]["ls_out"][:NPC]
    return (mu, ls), per_iter_ns, t_first


def kernel(**inputs):
    from concourse.bass_utils import run_bass_kernel_spmd

    plan, percore = preprocess(inputs["edge_index"], inputs["edge_weight"])
    in_maps = build_in_maps(inputs, plan, percore)
    key = (_plan_sig(plan), 5)
    if key not in _CACHE:
        _CACHE[key] = build_kernel(plan)
    nc = _CACHE[key]
    res = run_bass_kernel_spmd(nc, in_maps, core_ids=list(range(NC)))

    mu = np.zeros((N, P), np.float32)
    ls = np.zeros((N, P), np.float32)
    for c in range(NC):
        mu[c * NPC : (c + 1) * NPC] = res.results[c]["mu_out"][:NPC]
        ls[c * NPC : (c + 1) * NPC] = res.results[c]["ls_out"][:NPC]
    return (mu, ls)
